# revision 1
# baseline (speedup 1.0000x reference)
"""Self-contained Trainium2 Bass kernel for a single attention head.

Reference computation (per batch b):
    Q = x @ Wq + bq ; K = x @ Wk + bk ; V = x @ Wv + bv      (x: [S, M])
    out = softmax(Q K^T / sqrt(D)) @ V                        ([S, D])

Shapes: B=4, S=4096, M=1024, D=128, f32.

Sharding: 8 cores; core c handles batch b=c//2, query-half h=c%2 (2048 query
rows), with the full batch (4096 rows) as keys/values. Softmax is over the
key axis only, so key order is irrelevant: the host permutes each core's
batch so its own query rows come first, and pre-transposes to xT [M, S] so
the device needs no input transposes. No collectives.

Device layout (per core):
  - projections contract over M with fp32r matmuls: Q^T, K^T produced
    dk-major [128, s]; V produced naturally [s, 128] via 128x128 transposes.
  - scores computed transposed: S^T[s, q] = (K^T tile).T @ Q^T, fp32r,
    moving dim 512. exp (ACT engine) writes A^T bf16 - which is exactly the
    layout attn@V needs, so no O(S*S) transposes.
  - softmax denominator: level-1 pairwise bf16 adds on DVE (fast 2-byte
    mode), f32 wide accumulation -> [128, q] partials, one tiny f32
    ones-matmul -> [1, q], transpose + reciprocal -> per-q-row scale applied
    after the final O^T -> O transpose.
  - attn@V: O^T[dv, q] accumulated in PSUM over 32 bf16 matmuls.
  - fp32r trick: fp32r-typed DRAM inputs may feed fp32r matmuls directly
    (the BIR verifier accepts ExternalInput as pre-rounded), so x needs no
    on-device rounding pass. fp32r matmuls run at full PE rate for moving
    dims >= 256 with ~1.5e-4 error.
  - engine placement: PE matmuls ~113us (bottleneck), ACT exp+biases+psum
    copies ~94us, DVE den+normalize ~63us (cost-model estimates per core).
    GPSIMD must not touch PSUM (walrus restriction).
"""

from contextlib import ExitStack

import numpy as np

import concourse.bass as bass
import concourse.tile as tile
from concourse import bacc, mybir
from concourse.bass_utils import run_bass_kernel_spmd
from concourse.masks import make_identity

F32 = mybir.dt.float32
F32R = mybir.dt.float32r
BF16 = mybir.dt.bfloat16

B, S, M, D = 4, 4096, 1024, 128
N_CORES = 8
SCALE = 1.0 / np.sqrt(np.float32(D))


def build_attention(nc, S_keys=S, S_q=S // 2, M_dim=M, SC=512, QC=512,
                    repeat=1, phases=(1, 2), pair=False):
    """Emit the attention graph. S_keys: key rows; S_q: query rows (prefix of
    the permuted sequence); SC: phase-1 s-chunk; QC: phase-2 q-chunk.
    repeat>1 re-emits the whole body (for dispatch-free timing)."""
    P = 128
    MT = M_dim // P              # m-tiles
    ST = S_keys // P             # key s-tiles
    S_own = S_q if pair else S_keys   # key rows this core projects
    ST_own = S_own // P
    NSC = S_own // SC            # phase-1 chunks
    NSCQ = S_q // SC             # phase-1 chunks that also need Q
    NQC = S_q // QC              # phase-2 q-chunks
    SCT = SC // P                # 128-tiles per s-chunk
    QT = QC // P                 # 128-tiles per q-chunk
    GB = S_keys // S_own         # gather slots (2 in pair mode, else 1)
    assert not pair or GB == 2

    xT = nc.dram_tensor("xT", [M_dim, S_own], F32R, kind="ExternalInput").ap()
    wq = nc.dram_tensor("wq", [M_dim, D], F32R, kind="ExternalInput").ap()
    wk = nc.dram_tensor("wk", [M_dim, D], F32R, kind="ExternalInput").ap()
    wv = nc.dram_tensor("wv", [M_dim, D], F32R, kind="ExternalInput").ap()
    bq = nc.dram_tensor("bq", [D, 1], F32, kind="ExternalInput").ap()
    bk = nc.dram_tensor("bk", [D, 1], F32, kind="ExternalInput").ap()
    bv = nc.dram_tensor("bv", [D, 1], F32, kind="ExternalInput").ap()
    out = nc.dram_tensor("out", [S_q, D], F32, kind="ExternalOutput").ap()

    xT_r = xT.rearrange("(t p) s -> p t s", p=P)
    out_r = out.rearrange("(t p) d -> p t d", p=P)

    with tile.TileContext(nc) as tc:
      for _rep in range(repeat):
        ctx = ExitStack()
        persist = ctx.enter_context(tc.tile_pool(name="persist", bufs=1))

        ident = persist.tile([P, P], F32)
        make_identity(nc, ident[:])
        ident_r = persist.tile([P, P], F32R)
        nc.vector.tensor_copy(ident_r[:], ident[:])
        ones_col = persist.tile([P, 1], F32)
        nc.vector.memset(ones_col[:], 1.0)

        # weights/biases: fp32r-typed DRAM params DMA directly to fp32r tiles
        w_r = []
        b_sb = []
        for name, w_ap, b_ap in (("k", wk, bk), ("q", wq, bq), ("v", wv, bv)):
            wr = persist.tile([P, MT, D], F32R, name=f"w{name}_r")
            nc.scalar.dma_start(wr[:], w_ap.rearrange("(t p) d -> p t d", p=P))
            w_r.append(wr)
            bs = persist.tile([P, 1], F32, name=f"b{name}_sb")
            nc.scalar.dma_start(bs[:], b_ap)
            b_sb.append(bs)
        wk_r, wq_r, wv_r = w_r
        bk_sb, bq_sb, bv_sb = b_sb

        kT_sb = persist.tile([P, GB, S_own], F32R)  # K^T  [dk, slot, s]
        qT_sb = persist.tile([P, S_q], F32R)      # Q^T  [dk, q]
        v_sb = persist.tile([P, ST, D], BF16)     # V    [s%128, s-tile, dv]
        o_sb = persist.tile([P, S_q // P, D], F32)  # O   [q%128, q-tile, dv]

        def kt_tile(st):
            return kT_sb[:, st // ST_own, bass.ts(st % ST_own, P)]

        Ident = mybir.ActivationFunctionType.Identity
        Exp = mybir.ActivationFunctionType.Exp
        Copy = mybir.ActivationFunctionType.Copy

        # ---- phase 1: projections ----
        if 1 in phases:
          with (
            tc.tile_pool(name="xstage", bufs=(3 if SC <= 512 else 2)) as xstage,
            tc.tile_pool(name="vtmp", bufs=2) as vtmp,
            tc.tile_pool(name="dram", bufs=1, space="DRAM") as drampool,
            tc.tile_pool(name="p1psum", bufs=(2 if SC <= 512 else 1), space="PSUM") as p1psum,
            tc.tile_pool(name="p1tpsum", bufs=2, space="PSUM") as p1tpsum,
        ):
            if pair:
                # own-half staging; gathered via pair AllGather below
                kT_own = persist.tile([P, S_own], F32R)
                v_own = persist.tile([P, ST_own, D], BF16)
            else:
                kT_own = kT_sb[:, 0, :]
                v_own = v_sb

            for sc in range(NSC):
                ssl = bass.ds(sc * SC, SC)
                x_r = xstage.tile([P, MT, SC], F32R)
                (nc.sync if sc % 2 == 0 else nc.scalar).dma_start(
                    x_r[:], xT_r[:, :, ssl])

                # K^T chunk
                ps_k = p1psum.tile([P, SC], F32)
                for mt in range(MT):
                    nc.tensor.matmul(ps_k[:], wk_r[:, mt, :], x_r[:, mt, :],
                                     start=(mt == 0), stop=(mt == MT - 1))
                nc.scalar.activation(kT_own[:, ssl], ps_k[:], Ident,
                                     bias=bk_sb[:])

                # Q^T chunk (query rows are the permuted prefix)
                if sc < NSCQ:
                    ps_q = p1psum.tile([P, SC], F32)
                    for mt in range(MT):
                        nc.tensor.matmul(ps_q[:], wq_r[:, mt, :], x_r[:, mt, :],
                                         start=(mt == 0), stop=(mt == MT - 1))
                    nc.scalar.activation(qT_sb[:, ssl], ps_q[:], Ident,
                                         bias=bq_sb[:])

                # V^T chunk, then transpose to natural V tiles
                ps_v = p1psum.tile([P, SC], F32)
                for mt in range(MT):
                    nc.tensor.matmul(ps_v[:], wv_r[:, mt, :], x_r[:, mt, :],
                                     start=(mt == 0), stop=(mt == MT - 1))
                vt = vtmp.tile([P, SC], F32R)
                nc.scalar.activation(vt[:], ps_v[:], Ident, bias=bv_sb[:])
                for t in range(SCT):
                    ps_t = p1tpsum.tile([P, D], F32R)
                    nc.tensor.transpose(ps_t[:], vt[:, bass.ts(t, P)], ident_r[:])
                    nc.scalar.copy(v_own[:, sc * SCT + t, :], ps_t[:])

            if pair:
                groups = [[i, i + 1] for i in range(0, nc.num_devices, 2)]
                # K^T pair AllGather: [P, S_own] -> [2, P, S_own]
                kb = drampool.tile([P, S_own], F32R)
                kg = drampool.tile([GB * P, S_own], F32R)
                nc.sync.dma_start(kb[:], kT_own[:])
                nc.gpsimd.collective_compute(
                    "AllGather", mybir.AluOpType.bypass,
                    replica_groups=groups, ins=[kb.opt()], outs=[kg.opt()])
                nc.sync.dma_start(
                    kT_sb[:], kg.rearrange("(g p) s -> p g s", p=P))
                # V pair AllGather: [S_own, D] -> [2*S_own, D]
                vb = drampool.tile([S_own, D], BF16)
                vg = drampool.tile([GB * S_own, D], BF16)
                nc.sync.dma_start(vb.rearrange("(t p) d -> p t d", p=P), v_own[:])
                nc.gpsimd.collective_compute(
                    "AllGather", mybir.AluOpType.bypass,
                    replica_groups=groups, ins=[vb.opt()], outs=[vg.opt()])
                nc.sync.dma_start(
                    v_sb[:], vg.rearrange("(t p) d -> p t d", p=P))

        # ---- phase 2: attention ----
        if 2 in phases:
          with (
            tc.tile_pool(name="a_sb", bufs=2) as apool,
            tc.tile_pool(name="dacc", bufs=2) as dpool,
            tc.tile_pool(name="small", bufs=2 * QT) as small,
            tc.tile_pool(name="otmp", bufs=2) as otpool,
            tc.tile_pool(name="spsum", bufs=2, space="PSUM") as spsum,
            tc.tile_pool(name="opsum", bufs=1, space="PSUM") as opsum,
            tc.tile_pool(name="dpsum", bufs=1, space="PSUM") as dpsum,
            tc.tile_pool(name="otpsum", bufs=1, space="PSUM") as otpsum,
        ):
            for qc in range(NQC):
                qsl = bass.ds(qc * QC, QC)
                a_sb = apool.tile([P, ST, QC], BF16)
                # wide denominator accumulator: 4 q-chunk-wide lanes summed at
                # the end (fewer, larger DVE adds)
                DW = 4
                den4 = dpool.tile([P, DW, QC], F32)
                den = dpool.tile([P, QC], F32)

                # pass 1: scores (pairs) + one wide exp per pair
                assert ST % 2 == 0
                for sp in range(ST // 2):
                    ps_s = spsum.tile([P, 2, QC], F32)
                    for j in range(2):
                        nc.tensor.matmul(ps_s[:, j, :],
                                         kt_tile(2 * sp + j),
                                         qT_sb[:, qsl], start=True, stop=True)
                    nc.scalar.activation(a_sb[:, 2 * sp:2 * sp + 2, :], ps_s[:],
                                         Exp, scale=float(SCALE))
                # denominator partials: level-1 pairwise bf16 adds (4x DVE mode),
                # then f32 wide accumulation of the 16 pair-sums
                assert ST % 2 == 0
                npair = ST // 2
                apair = dpool.tile([P, npair, QC], BF16)
                for pr in range(npair):
                    nc.vector.tensor_add(apair[:, pr, :], a_sb[:, 2 * pr, :],
                                         a_sb[:, 2 * pr + 1, :])
                if npair % DW == 0:
                    for g in range(npair // DW):
                        grp = apair[:, g * DW:(g + 1) * DW, :]
                        if g == 0:
                            nc.vector.tensor_copy(den4[:], grp)
                        else:
                            nc.vector.tensor_add(den4[:], den4[:], grp)
                    nc.vector.tensor_add(den4[:, 0, :], den4[:, 0, :], den4[:, 1, :])
                    nc.vector.tensor_add(den4[:, 2, :], den4[:, 2, :], den4[:, 3, :])
                    nc.vector.tensor_add(den[:], den4[:, 0, :], den4[:, 2, :])
                else:
                    nc.vector.tensor_copy(den4[:, :npair, :], apair[:])
                    for pr in range(1, npair):
                        nc.vector.tensor_add(den4[:, 0, :], den4[:, 0, :],
                                             den4[:, pr, :])
                    nc.vector.tensor_copy(den[:], den4[:, 0, :])

                # pass 2: O^T accumulation
                ps_o = opsum.tile([P, QC], F32)
                for st in range(ST):
                    nc.tensor.matmul(ps_o[:], v_sb[:, st, :], a_sb[:, st, :],
                                     start=(st == 0), stop=(st == ST - 1))
                oT = otpool.tile([P, QC], F32)
                nc.scalar.copy(oT[:], ps_o[:])

                # denominator: [128, QC] -> [1, QC] -> transpose -> reciprocal
                ps_d = dpsum.tile([1, QC], F32)
                nc.tensor.matmul(ps_d[:], ones_col[:], den[:], start=True, stop=True)
                den_flat = small.tile([1, QC], F32)
                nc.scalar.copy(den_flat[:], ps_d[:])

                for t in range(QT):
                    ps_dt = dpsum.tile([P, 1], F32)
                    nc.tensor.transpose(ps_dt[:], den_flat[:1, bass.ts(t, P)],
                                        ident[:1, :1])
                    rden = small.tile([P, 1], F32)
                    nc.vector.reciprocal(rden[:], ps_dt[:])
                    ps_ot = otpsum.tile([P, D], F32)
                    nc.tensor.transpose(ps_ot[:], oT[:, bass.ts(t, P)], ident[:])
                    nc.vector.tensor_scalar_mul(o_sb[:, qc * QT + t, :],
                                                ps_ot[:], rden[:])
                (nc.sync if qc % 2 == 0 else nc.scalar).dma_start(
                    out_r[:, qc * QT:(qc + 1) * QT, :],
                    o_sb[:, qc * QT:(qc + 1) * QT, :])
        ctx.close()

    return nc


def build(n_cores=N_CORES, **kw):
    nc = bacc.Bacc("TRN2", target_bir_lowering=False, debug=False,
                   num_devices=n_cores)
    build_attention(nc, **kw)
    nc.compile()
    return nc


PAIR = False


def shard_inputs(input, Wq, bq, Wk, bk, Wv, bv, pair=PAIR):
    """Build per-core in_maps. Core c: batch c//2, query-half c%2. In pair
    mode each core only gets its own half (K/V gathered on-device); otherwise
    it gets the whole batch with its query rows permuted to the front
    (softmax is key-permutation invariant)."""
    half = S // 2
    in_maps = []
    for c in range(N_CORES):
        b, h = divmod(c, 2)
        xb = np.asarray(input[b])
        if pair:
            x_perm = xb[h * half:(h + 1) * half]
        else:
            x_perm = np.concatenate(
                [xb[h * half:(h + 1) * half], xb[(1 - h) * half:(2 - h) * half]],
                axis=0)
        in_maps.append({
            "xT": np.ascontiguousarray(x_perm.T, dtype=np.float32),
            "wq": np.asarray(Wq, dtype=np.float32),
            "wk": np.asarray(Wk, dtype=np.float32),
            "wv": np.asarray(Wv, dtype=np.float32),
            "bq": np.asarray(bq, dtype=np.float32).reshape(D, 1),
            "bk": np.asarray(bk, dtype=np.float32).reshape(D, 1),
            "bv": np.asarray(bv, dtype=np.float32).reshape(D, 1),
        })
    return in_maps


_NC_CACHE = {}


def kernel(input, Wq, bq, Wk, bk, Wv, bv):
    in_maps = shard_inputs(input, Wq, bq, Wk, bk, Wv, bv)
    if "nc" not in _NC_CACHE:
        _NC_CACHE["nc"] = build(pair=PAIR)
    nc = _NC_CACHE["nc"]
    res = run_bass_kernel_spmd(nc, in_maps, core_ids=list(range(N_CORES)))
    half = S // 2
    result = np.empty((B, S, D), dtype=np.float32)
    for c in range(N_CORES):
        b, h = divmod(c, 2)
        result[b, h * half:(h + 1) * half] = res.results[c]["out"]
    return result


if __name__ == "__main__":
    rng = np.random.default_rng(0)
    inputs = {
        "input": rng.standard_normal((B, S, M), dtype=np.float32),
        "Wq": (rng.standard_normal((M, D), dtype=np.float32) / np.sqrt(M)).astype(np.float32),
        "bq": (rng.standard_normal(D, dtype=np.float32) * 0.02),
        "Wk": (rng.standard_normal((M, D), dtype=np.float32) / np.sqrt(M)).astype(np.float32),
        "bk": (rng.standard_normal(D, dtype=np.float32) * 0.02),
        "Wv": (rng.standard_normal((M, D), dtype=np.float32) / np.sqrt(M)).astype(np.float32),
        "bv": (rng.standard_normal(D, dtype=np.float32) * 0.02),
    }
    out = kernel(**inputs)
    print("kernel output:", out.shape, out.dtype)



# revision 23
# speedup vs baseline: 1.2535x; 1.2535x over previous
"""Self-contained Trainium2 Bass kernel for a single attention head.

Reference computation (per batch b):
    Q = x @ Wq + bq ; K = x @ Wk + bk ; V = x @ Wv + bv      (x: [S, M])
    out = softmax(Q K^T / sqrt(D)) @ V                        ([S, D])

Shapes: B=4, S=4096, M=1024, D=128, f32.

Sharding: 8 cores; core c handles batch b=c//2, query-half h=c%2 (2048 query
rows), with the full batch (4096 rows) as keys/values. Softmax is over the
key axis only, so key order is irrelevant: the host permutes each core's
batch so its own query rows come first, pre-transposes to xT [M, S], and
converts to bf16 (halves input DMA; rel-err budget measured ~4e-3).

Device pipeline (fully fused, no phase barrier):
  - stream 8 s-chunks (512 keys): K^T/Q^T chunks via bf16 matmuls (moving
    dim 512) + ACT bias-drains; V in natural [s, dv] layout directly via
    x-tile-stationary matmuls (no transposes) + DVE bias-drains.
  - q-chunks 0/1 chase the stream: scores S^T[s,q] (bf16, PSUM), one wide
    exp per pair [128,1024] -> A^T bf16 (exactly the attn@V layout), attn@V
    accumulates O^T in PSUM across all 32 s-tiles as they appear.
  - q-chunks 2/3 run after the stream from resident K^T/V.
  - softmax denominator: DVE bf16 wide adds (2x mode) -> f32 fold ->
    GPSIMD partition_all_reduce; for the last q-chunk the final 8 s-tiles
    accumulate on the PE via ones-matmuls so the post-exp tail is short.
  - finalize per qc: O^T -> bf16 -> PE transpose -> DVE scale by 1/den ->
    bf16 out DMA (host converts to f32).
  - PSUM budget 8 banks: proj pool 2 + scores 4 + O^T/tail 2, finalize
    pool reuses the closed proj pool's banks.
"""

from contextlib import ExitStack

import numpy as np

import concourse.bass as bass
import concourse.tile as tile
from concourse import bacc, mybir
from concourse.bass_utils import run_bass_kernel_spmd
from concourse.masks import make_identity

F32 = mybir.dt.float32
BF16 = mybir.dt.bfloat16

B, S, M, D = 4, 4096, 1024, 128
N_CORES = 8
P = 128
SCALE = 1.0 / np.sqrt(np.float32(D))


def build_attention(nc, S_keys=S, S_q=S // 2, M_dim=M, SC=512, QC=512):
    MT = M_dim // P               # m-tiles (8)
    ST = S_keys // P              # key s-tiles (32)
    NSC = S_keys // SC            # s-chunks (8)
    SCT = SC // P                 # s-tiles per chunk (4)
    NQC = S_q // QC               # q-chunks (4)
    QT = QC // P                  # q-tiles per q-chunk (4)
    NCH = min(2, NQC)             # q-chunks chasing the projection stream
    NPAIR = ST // 2               # score pairs per q-chunk (16)

    # weights host-packed [P, MT*D] (contiguous per partition), biases packed
    # into one [P, 2] (bk|bq) + one [1, D] (bv) tensor so their DMAs are tiny
    # and early.
    xT = nc.dram_tensor("xT", [M_dim, S_keys], BF16, kind="ExternalInput").ap()
    wq = nc.dram_tensor("wq", [P, MT * D], BF16, kind="ExternalInput").ap()
    wk = nc.dram_tensor("wk", [P, MT * D], BF16, kind="ExternalInput").ap()
    wv = nc.dram_tensor("wv", [P, MT * D], BF16, kind="ExternalInput").ap()
    bkq = nc.dram_tensor("bkq", [P, 2], F32, kind="ExternalInput").ap()
    bv = nc.dram_tensor("bv", [1, D], F32, kind="ExternalInput").ap()
    out = nc.dram_tensor("out", [S_q, D], BF16, kind="ExternalOutput").ap()

    xT_r = xT.rearrange("(t p) s -> p t s", p=P)
    out_r = out.rearrange("(t p) d -> p t d", p=P)

    Ident = mybir.ActivationFunctionType.Identity
    Exp = mybir.ActivationFunctionType.Exp

    with tile.TileContext(nc) as tc:
        ctx = ExitStack()
        persist = ctx.enter_context(tc.tile_pool(name="persist", bufs=1))

        ident = persist.tile([P, P], F32)
        make_identity(nc, ident[:])
        identb = persist.tile([P, P], BF16)
        nc.vector.tensor_copy(identb[:], ident[:])
        ones_f = persist.tile([P, 1], F32)
        nc.vector.memset(ones_f[:], 1.0)
        onesb = persist.tile([P, 1], BF16)
        nc.vector.tensor_copy(onesb[:], ones_f[:])

        # DMA issue order matters at startup (the engine serializes transfers
        # in issue order; the gpsimd queue dispatches fastest): wk, then x
        # chunk 0 in halves, then wq/wv (chunk-0 Q/V projections), then the
        # chunk-1 prefetch, then biases.
        xstage = ctx.enter_context(tc.tile_pool(name="xstage", bufs=3))
        wk_sb = persist.tile([P, MT, D], BF16)
        nc.gpsimd.dma_start(wk_sb[:], wk.rearrange("p (t d) -> p t d", d=D))
        bkq_sb = persist.tile([P, 2], F32)
        nc.sync.dma_start(bkq_sb[:], bkq)
        bk_sb = bkq_sb[:, 0:1]
        bq_sb = bkq_sb[:, 1:2]
        bv_row = persist.tile([1, D], F32)
        nc.sync.dma_start(bv_row[:], bv)
        bv_bcast = persist.tile([P, D], F32)
        nc.gpsimd.partition_broadcast(bv_bcast[:], bv_row[:])
        x_r0 = xstage.tile([P, MT, SC], BF16, name="x_r")
        nc.sync.dma_start(x_r0[:, 0:MT // 2, :],
                          xT_r[:, 0:MT // 2, bass.ds(0, SC)])
        nc.sync.dma_start(x_r0[:, MT // 2:, :],
                          xT_r[:, MT // 2:, bass.ds(0, SC)])
        wq_sb = persist.tile([P, MT, D], BF16)
        nc.sync.dma_start(wq_sb[:], wq.rearrange("p (t d) -> p t d", d=D))
        wv_sb = persist.tile([P, MT, D], BF16)
        nc.sync.dma_start(wv_sb[:], wv.rearrange("p (t d) -> p t d", d=D))

        kT_sb = persist.tile([P, S_keys], BF16)    # K^T  [dk, s]
        qT_sb = persist.tile([P, S_q], BF16)       # Q^T  [dk, q]
        v_sb = persist.tile([P, ST, D], BF16)      # V    [s%128, s-tile, dv]
        o_sb = persist.tile([P, S_q // P, D], BF16)  # O   [q%128, q-tile, dv]

        apool = ctx.enter_context(tc.tile_pool(name="apool", bufs=2))
        dpool = ctx.enter_context(tc.tile_pool(name="dpool", bufs=2))
        otpool = ctx.enter_context(tc.tile_pool(name="otpool", bufs=2))
        spsum = ctx.enter_context(tc.tile_pool(name="spsum", bufs=2, space="PSUM"))
        opsum = ctx.enter_context(tc.tile_pool(name="opsum", bufs=2, space="PSUM"))

        a_t = {}      # qc -> A^T tile [P, ST, QC] bf16
        o_ps = {}     # qc -> O^T psum [P, QC]
        t1a = {}      # qc -> den partials [P, 4, QC] bf16 (tiles 0,8,16.. lanes)
        t1b = {}
        den_all = {}  # qc -> all-reduced denominator [P, QC] f32
        tail = {}     # tail-mode PE-side den psum [1, QC]

        def qsl(qc):
            return bass.ds(qc * QC, QC)

        def emit_scores(qc, pr):
            """Scores for s-tiles (2pr, 2pr+1) x q-chunk qc + one wide exp."""
            if pr == 0:
                a_t[qc] = apool.tile([P, ST, QC], BF16, name="a_sb")
                o_ps[qc] = opsum.tile([P, QC], F32, name="o_ps")
            ps_s = spsum.tile([P, 2, QC], F32, name="ps_s")
            for j in range(2):
                st = 2 * pr + j
                nc.tensor.matmul(ps_s[:, j, :], kT_sb[:, bass.ts(st, P)],
                                 qT_sb[:, qsl(qc)], start=True, stop=True)
            nc.scalar.activation(a_t[qc][:, 2 * pr:2 * pr + 2, :], ps_s[:],
                                 Exp, scale=float(SCALE))

        def emit_av(qc, pr, tail_mode=False):
            """attn@V accumulation matmuls for pair pr (+ the PE ones-matmul
            denominator tail in tail mode)."""
            for j in range(2):
                st = 2 * pr + j
                nc.tensor.matmul(o_ps[qc][:], v_sb[:, st, :],
                                 a_t[qc][:, st, :],
                                 start=(st == 0), stop=(st == ST - 1))
            if tail_mode and pr >= 12:
                if pr == 12:
                    tail[qc] = opsum.tile([1, QC], F32, name="o_ps")
                for j in range(2):
                    st = 2 * pr + j
                    nc.tensor.matmul(tail[qc][:], onesb[:], a_t[qc][:, st, :],
                                     start=(st == 24), stop=(st == ST - 1))

        def emit_pair(qc, pr):
            emit_scores(qc, pr)
            emit_av(qc, pr)

        def emit_den(qc, pr, tail_mode):
            """Denominator pieces as A^T tiles become available. DVE adds are
            kept to [P,4,QC] so they can't head-of-line-block V drains long.
            Tree mode: all 32 tiles on DVE + f32 fold + AR.
            Tail mode: tiles 0..23 on DVE (folded at pr11), tiles 24..31 on
            PE ones-matmuls so the post-last-exp latency is tiny."""
            a = a_t[qc]
            if pr == 7:
                t1a[qc] = dpool.tile([P, 4, QC], BF16, name="t1a")
                t1b[qc] = dpool.tile([P, 4, QC], BF16, name="t1b")
                nc.vector.tensor_add(t1a[qc][:], a[:, 0:4, :], a[:, 4:8, :])
                nc.vector.tensor_add(t1b[qc][:], a[:, 8:12, :], a[:, 12:16, :])
            elif pr == 11:
                nc.vector.tensor_add(t1a[qc][:], t1a[qc][:], a[:, 16:20, :])
                nc.vector.tensor_add(t1b[qc][:], t1b[qc][:], a[:, 20:24, :])
                if tail_mode:
                    _fold_and_reduce(qc)
            elif pr == 15 and not tail_mode:
                nc.vector.tensor_add(t1a[qc][:], t1a[qc][:], a[:, 24:28, :])
                nc.vector.tensor_add(t1b[qc][:], t1b[qc][:], a[:, 28:32, :])
                _fold_and_reduce(qc)

        def _fold_and_reduce(qc):
            f1 = dpool.tile([P, 4, QC], F32, name="f1")
            nc.vector.tensor_add(f1[:], t1a[qc][:], t1b[qc][:])
            nc.vector.tensor_add(f1[:, 0:2, :], f1[:, 0:2, :], f1[:, 2:4, :])
            den128 = dpool.tile([P, QC], F32, name="den128")
            nc.vector.tensor_add(den128[:], f1[:, 0, :], f1[:, 1, :])
            dall = dpool.tile([P, QC], F32, name="den_all")
            nc.gpsimd.partition_all_reduce(dall[:], den128[:], P,
                                           bass.bass_isa.ReduceOp.add)
            den_all[qc] = dall

        def emit_drain(qc):
            """O^T psum -> bf16 SBUF; frees the opsum slot."""
            oT = otpool.tile([P, QC], BF16, name="oT")
            nc.vector.tensor_copy(oT[:], o_ps[qc][:])
            return oT

        oT_sb = {}

        def emit_finalize(qc, fin, tail_mode):
            """Per q-tile: denominator transpose + reciprocal, O^T transpose,
            scale, and the output DMA."""
            if tail_mode:
                tail_sb = dpool.tile([1, QC], F32, name="tail_sb")
                nc.vector.tensor_copy(tail_sb[:], tail[qc][:])
                dcomb = dpool.tile([1, QC], F32, name="dcomb")
                nc.vector.tensor_add(dcomb[:], den_all[qc][:1, :], tail_sb[:])
                dsrc = dcomb
            else:
                dsrc = den_all[qc]
            oT = oT_sb[qc]
            for t in range(QT):
                ps_dt = fin.tile([P, 1], F32, name="fin_t")
                nc.tensor.transpose(ps_dt[:], dsrc[:1, bass.ts(t, P)],
                                    ident[:1, :1])
                rden = dpool.tile([P, 1], F32, name="rden", bufs=2 * QT)
                nc.vector.reciprocal(rden[:], ps_dt[:])
                ps_ot = fin.tile([P, D], BF16, name="fin_t")
                nc.tensor.transpose(ps_ot[:], oT[:, bass.ts(t, P)], identb[:])
                nc.vector.tensor_scalar_mul(o_sb[:, qc * QT + t, :],
                                            ps_ot[:], rden[:])
            nc.sync.dma_start(out_r[:, qc * QT:(qc + 1) * QT, :],
                              o_sb[:, qc * QT:(qc + 1) * QT, :])

        # PE warm-up: ~3.3us of back-to-back transposes ramps the PE to its
        # full 2.4 GHz p-state before the first real matmul arrives.
        with tc.tile_pool(name="warm", bufs=1, space="PSUM") as wp:
            warm_ps = wp.tile([P, P], BF16, name="warm_ps")
            for _ in range(34):
                nc.tensor.transpose(warm_ps[:], identb[:], identb[:])

        # ---- streaming phase: projections + chasing q-chunks 0..NCH-1 ----
        pend = [0] * NCH
        with tc.tile_pool(name="pp", bufs=2, space="PSUM") as pp:
            for sc in range(NSC):
                ssl = bass.ds(sc * SC, SC)
                if sc == 0:
                    x_r = x_r0
                else:
                    x_r = xstage.tile([P, MT, SC], BF16, name="x_r")
                    nc.sync.dma_start(x_r[:], xT_r[:, :, ssl])

                # K^T chunk
                ps = pp.tile([P, SC], F32, name="pp")
                for mt in range(MT):
                    nc.tensor.matmul(ps[:], wk_sb[:, mt, :], x_r[:, mt, :],
                                     start=(mt == 0), stop=(mt == MT - 1))
                nc.scalar.activation(kT_sb[:, ssl], ps[:], Ident, bias=bk_sb)

                # Q^T chunk (query rows are the permuted prefix)
                if sc * SC < S_q:
                    ps2 = pp.tile([P, SC], F32, name="pp")
                    for mt in range(MT):
                        nc.tensor.matmul(ps2[:], wq_sb[:, mt, :], x_r[:, mt, :],
                                         start=(mt == 0), stop=(mt == MT - 1))
                    nc.scalar.activation(qT_sb[:, ssl], ps2[:], Ident,
                                         bias=bq_sb)

                # V chunk in natural [s, dv] layout: x-tile stationary
                for t in range(SCT):
                    st = sc * SCT + t
                    psv = pp.tile([P, D], F32, name="pp")
                    for mt in range(MT):
                        nc.tensor.matmul(psv[:], x_r[:, mt, bass.ts(t, P)],
                                         wv_sb[:, mt, :],
                                         start=(mt == 0), stop=(mt == MT - 1))
                    nc.vector.tensor_add(v_sb[:, st, :], psv[:], bv_bcast[:])

                # chasing q-chunks catch up on all available pairs
                avail = (sc + 1) * SCT // 2
                for qc in range(NCH):
                    if (qc + 1) * QC <= (sc + 1) * SC:
                        while pend[qc] < avail:
                            pr = pend[qc]
                            emit_pair(qc, pr)
                            emit_den(qc, pr, tail_mode=False)
                            pend[qc] += 1

        # ---- post-stream: drains, remaining q-chunks, finalize ----
        # The chasers' den chains complete ~8us after the stream ends, so
        # their finalizes are scheduled deep into qc2/qc3. qc2/qc3 use
        # tail-mode dens (PE ones-matmuls for the last 8 tiles) so their own
        # finalizes are prompt.
        with tc.tile_pool(name="fin", bufs=2, space="PSUM") as fin:
            for qc in range(NCH):
                oT_sb[qc] = emit_drain(qc)

            fin_sched = {}  # qc -> [(after_pair, qc_to_finalize)]
            if NQC > NCH:
                fin_sched[2] = [(12, 0)]
                fin_sched[3] = [(3, 1), (8, 2)]
            else:
                fin_sched[1] = [(8, 0)]

            # AV (and the ones-matmul den tail) lag scores by one pair so the
            # PE never waits on the exp it just issued.
            for qc in range(NCH, NQC):
                last = qc == NQC - 1
                for pr in range(NPAIR):
                    emit_scores(qc, pr)
                    if pr > 0:
                        emit_av(qc, pr - 1, tail_mode=True)
                        emit_den(qc, pr - 1, tail_mode=True)
                    for (after, fqc) in fin_sched.get(qc, []):
                        if pr == after:
                            emit_finalize(fqc, fin,
                                          tail_mode=(fqc >= NCH))
                emit_av(qc, NPAIR - 1, tail_mode=True)
                emit_den(qc, NPAIR - 1, tail_mode=True)
                if not last:
                    oT_sb[qc] = emit_drain(qc)

            lq = NQC - 1
            oT_sb[lq] = emit_drain(lq)
            if NQC == NCH:
                emit_finalize(lq - 1, fin, tail_mode=False)
            emit_finalize(lq, fin, tail_mode=True)
        ctx.close()

    return nc


def build(n_cores=N_CORES, **kw):
    nc = bacc.Bacc("TRN2", target_bir_lowering=False, debug=False,
                   num_devices=n_cores)
    build_attention(nc, **kw)
    nc.compile()
    return nc


def shard_inputs(input, Wq, bq, Wk, bk, Wv, bv):
    """Per-core in_maps. Core c: batch c//2, query-half c%2; the host permutes
    the batch so the core's query rows come first (softmax is key-permutation
    invariant), transposes to xT [M, S], and converts to bf16."""
    import ml_dtypes
    half = S // 2
    MT = M // 128

    def pack_w(W):
        # [M, D] -> [P, MT*D] with [p, mt*D + d] = W[mt*128 + p, d]
        return np.ascontiguousarray(
            np.asarray(W, dtype=np.float32).reshape(MT, 128, D)
            .transpose(1, 0, 2).reshape(128, MT * D)).astype(ml_dtypes.bfloat16)

    wq_b, wk_b, wv_b = pack_w(Wq), pack_w(Wk), pack_w(Wv)
    bkq_f = np.ascontiguousarray(np.stack(
        [np.asarray(bk, dtype=np.float32).ravel(),
         np.asarray(bq, dtype=np.float32).ravel()], axis=1))
    bv_f = np.asarray(bv, dtype=np.float32).reshape(1, D)
    in_maps = []
    for c in range(N_CORES):
        b, h = divmod(c, 2)
        xb = np.asarray(input[b])
        x_perm = np.concatenate(
            [xb[h * half:(h + 1) * half], xb[(1 - h) * half:(2 - h) * half]],
            axis=0)
        xT = np.ascontiguousarray(x_perm.T).astype(ml_dtypes.bfloat16)
        in_maps.append({
            "xT": xT,
            "wq": wq_b, "wk": wk_b, "wv": wv_b,
            "bkq": bkq_f, "bv": bv_f,
        })
    return in_maps


_NC_CACHE = {}


def kernel(input, Wq, bq, Wk, bk, Wv, bv):
    in_maps = shard_inputs(input, Wq, bq, Wk, bk, Wv, bv)
    if "nc" not in _NC_CACHE:
        _NC_CACHE["nc"] = build()
    nc = _NC_CACHE["nc"]
    res = run_bass_kernel_spmd(nc, in_maps, core_ids=list(range(N_CORES)))
    half = S // 2
    result = np.empty((B, S, D), dtype=np.float32)
    for c in range(N_CORES):
        b, h = divmod(c, 2)
        result[b, h * half:(h + 1) * half] = np.asarray(
            res.results[c]["out"]).astype(np.float32)
    return result


if __name__ == "__main__":
    rng = np.random.default_rng(0)
    inputs = {
        "input": rng.standard_normal((B, S, M), dtype=np.float32),
        "Wq": (rng.standard_normal((M, D), dtype=np.float32) / np.sqrt(M)).astype(np.float32),
        "bq": (rng.standard_normal(D, dtype=np.float32) * 0.02),
        "Wk": (rng.standard_normal((M, D), dtype=np.float32) / np.sqrt(M)).astype(np.float32),
        "bk": (rng.standard_normal(D, dtype=np.float32) * 0.02),
        "Wv": (rng.standard_normal((M, D), dtype=np.float32) / np.sqrt(M)).astype(np.float32),
        "bv": (rng.standard_normal(D, dtype=np.float32) * 0.02),
    }
    out = kernel(**inputs)
    print("kernel output:", out.shape, out.dtype)


# revision 29
# speedup vs baseline: 1.2911x; 1.0300x over previous
"""Self-contained Trainium2 Bass kernel for a single attention head.

Reference computation (per batch b):
    Q = x @ Wq + bq ; K = x @ Wk + bk ; V = x @ Wv + bv      (x: [S, M])
    out = softmax(Q K^T / sqrt(D)) @ V                        ([S, D])

Shapes: B=4, S=4096, M=1024, D=128, f32.

Sharding: 8 cores; core c handles batch b=c//2, query-half h=c%2 (2048 query
rows), with the full batch (4096 rows) as keys/values. Softmax is over the
key axis only, so key order is irrelevant: the host permutes each core's
batch so its own query rows come first, pre-transposes to xT [M, S], and
converts to bf16 (halves input DMA; rel-err budget measured ~4e-3).

Device pipeline (fully fused, no phase barrier):
  - stream 8 s-chunks (512 keys): K^T/Q^T chunks via bf16 matmuls (moving
    dim 512) + ACT bias-drains; V in natural [s, dv] layout directly via
    x-tile-stationary matmuls (no transposes) + DVE bias-drains.
  - q-chunks 0/1 chase the stream: scores S^T[s,q] (bf16, PSUM), one wide
    exp per pair [128,1024] -> A^T bf16 (exactly the attn@V layout), attn@V
    accumulates O^T in PSUM across all 32 s-tiles as they appear.
  - q-chunks 2/3 run after the stream from resident K^T/V.
  - softmax denominator: DVE bf16 wide adds (2x mode) -> f32 fold ->
    GPSIMD partition_all_reduce; for the last q-chunk the final 8 s-tiles
    accumulate on the PE via ones-matmuls so the post-exp tail is short.
  - finalize per qc: O^T -> bf16 -> PE transpose -> DVE scale by 1/den ->
    bf16 out DMA (host converts to f32).
  - PSUM budget 8 banks: proj pool 2 + scores 4 + O^T/tail 2, finalize
    pool reuses the closed proj pool's banks.
"""

from contextlib import ExitStack

import numpy as np

import concourse.bass as bass
import concourse.tile as tile
from concourse import bacc, mybir
from concourse.bass_utils import run_bass_kernel_spmd
from concourse.masks import make_identity

F32 = mybir.dt.float32
BF16 = mybir.dt.bfloat16

B, S, M, D = 4, 4096, 1024, 128
N_CORES = 8
P = 128
SCALE = 1.0 / np.sqrt(np.float32(D))


def build_attention(nc, S_keys=S, S_q=S // 2, M_dim=M, SC=512, QC=512):
    MT = M_dim // P               # m-tiles (8)
    ST = S_keys // P              # key s-tiles (32)
    NSC = S_keys // SC            # s-chunks (8)
    SCT = SC // P                 # s-tiles per chunk (4)
    NQC = S_q // QC               # q-chunks (4)
    QT = QC // P                  # q-tiles per q-chunk (4)
    NCH = min(3, NQC - 1)         # q-chunks chasing the projection stream
    FULL_CH = min(2, NCH)         # chasers that also run attn@V in-stream
    NPAIR = ST // 2               # score pairs per q-chunk (16)

    # weights host-packed [P, MT*D] (contiguous per partition), biases packed
    # into one [P, 2] (bk|bq) + one [1, D] (bv) tensor so their DMAs are tiny
    # and early.
    xT = nc.dram_tensor("xT", [M_dim, S_keys], BF16, kind="ExternalInput").ap()
    wq = nc.dram_tensor("wq", [P, MT * D], BF16, kind="ExternalInput").ap()
    wk = nc.dram_tensor("wk", [P, MT * D], BF16, kind="ExternalInput").ap()
    wv = nc.dram_tensor("wv", [P, MT * D], BF16, kind="ExternalInput").ap()
    bkq = nc.dram_tensor("bkq", [P, 2], F32, kind="ExternalInput").ap()
    bv = nc.dram_tensor("bv", [1, D], F32, kind="ExternalInput").ap()
    out = nc.dram_tensor("out", [S_q, D], BF16, kind="ExternalOutput").ap()

    xT_r = xT.rearrange("(t p) s -> p t s", p=P)
    out_r = out.rearrange("(t p) d -> p t d", p=P)

    Ident = mybir.ActivationFunctionType.Identity
    Exp = mybir.ActivationFunctionType.Exp

    with tile.TileContext(nc) as tc:
        ctx = ExitStack()
        persist = ctx.enter_context(tc.tile_pool(name="persist", bufs=1))

        ident = persist.tile([P, P], F32)
        make_identity(nc, ident[:])
        identb = persist.tile([P, P], BF16)
        nc.vector.tensor_copy(identb[:], ident[:])
        ones_f = persist.tile([P, 1], F32)
        nc.vector.memset(ones_f[:], 1.0)
        onesb = persist.tile([P, 1], BF16)
        nc.vector.tensor_copy(onesb[:], ones_f[:])

        # DMA issue order matters at startup (the engine serializes transfers
        # in issue order; the gpsimd queue dispatches fastest): wk, then x
        # chunk 0 in halves, then wq/wv (chunk-0 Q/V projections), then the
        # chunk-1 prefetch, then biases.
        xstage = ctx.enter_context(tc.tile_pool(name="xstage", bufs=2))
        wk_sb = persist.tile([P, MT, D], BF16)
        nc.gpsimd.dma_start(wk_sb[:], wk.rearrange("p (t d) -> p t d", d=D))
        bkq_sb = persist.tile([P, 2], F32)
        nc.sync.dma_start(bkq_sb[:], bkq)
        bk_sb = bkq_sb[:, 0:1]
        bq_sb = bkq_sb[:, 1:2]
        bv_row = persist.tile([1, D], F32)
        nc.sync.dma_start(bv_row[:], bv)
        bv_bcast = persist.tile([P, D], F32)
        nc.gpsimd.partition_broadcast(bv_bcast[:], bv_row[:])
        x_r0 = xstage.tile([P, MT, SC], BF16, name="x_r")
        nc.sync.dma_start(x_r0[:, 0:MT // 2, :],
                          xT_r[:, 0:MT // 2, bass.ds(0, SC)])
        nc.sync.dma_start(x_r0[:, MT // 2:, :],
                          xT_r[:, MT // 2:, bass.ds(0, SC)])
        wq_sb = persist.tile([P, MT, D], BF16)
        nc.sync.dma_start(wq_sb[:], wq.rearrange("p (t d) -> p t d", d=D))
        wv_sb = persist.tile([P, MT, D], BF16)
        nc.sync.dma_start(wv_sb[:], wv.rearrange("p (t d) -> p t d", d=D))

        kT_sb = persist.tile([P, S_keys], BF16)    # K^T  [dk, s]
        qT_sb = persist.tile([P, S_q], BF16)       # Q^T  [dk, q]
        v_sb = persist.tile([P, ST, D], BF16)      # V    [s%128, s-tile, dv]
        o_sb = persist.tile([P, S_q // P, D], BF16)  # O   [q%128, q-tile, dv]

        apool = ctx.enter_context(tc.tile_pool(name="apool", bufs=3))
        dpool = ctx.enter_context(tc.tile_pool(name="dpool", bufs=2))
        otpool = ctx.enter_context(tc.tile_pool(name="otpool", bufs=2))
        spsum = ctx.enter_context(tc.tile_pool(name="spsum", bufs=2, space="PSUM"))
        opsum = ctx.enter_context(tc.tile_pool(name="opsum", bufs=2, space="PSUM"))

        a_t = {}      # qc -> A^T tile [P, ST, QC] bf16
        o_ps = {}     # qc -> O^T psum [P, QC]
        t1a = {}      # qc -> den partials [P, 4, QC] bf16 (tiles 0,8,16.. lanes)
        t1b = {}
        den_all = {}  # qc -> all-reduced denominator [P, QC] f32
        tail = {}     # tail-mode PE-side den psum [1, QC]

        def qsl(qc):
            return bass.ds(qc * QC, QC)

        def emit_scores(qc, pr):
            """Scores for s-tiles (2pr, 2pr+1) x q-chunk qc + one wide exp."""
            if pr == 0:
                a_t[qc] = apool.tile([P, ST, QC], BF16, name="a_sb")
            ps_s = spsum.tile([P, 2, QC], F32, name="ps_s")
            for j in range(2):
                st = 2 * pr + j
                nc.tensor.matmul(ps_s[:, j, :], kT_sb[:, bass.ts(st, P)],
                                 qT_sb[:, qsl(qc)], start=True, stop=True)
            nc.scalar.activation(a_t[qc][:, 2 * pr:2 * pr + 2, :], ps_s[:],
                                 Exp, scale=float(SCALE))

        def emit_av(qc, pr, tail_mode=False):
            """attn@V accumulation matmuls for pair pr (+ the PE ones-matmul
            denominator tail in tail mode)."""
            if qc not in o_ps:
                o_ps[qc] = opsum.tile([P, QC], F32, name="o_ps")
            for j in range(2):
                st = 2 * pr + j
                nc.tensor.matmul(o_ps[qc][:], v_sb[:, st, :],
                                 a_t[qc][:, st, :],
                                 start=(st == 0), stop=(st == ST - 1))
            if tail_mode and pr >= 12:
                if pr == 12:
                    tail[qc] = opsum.tile([1, QC], F32, name="o_ps")
                for j in range(2):
                    st = 2 * pr + j
                    nc.tensor.matmul(tail[qc][:], onesb[:], a_t[qc][:, st, :],
                                     start=(st == 24), stop=(st == ST - 1))

        # den fold engine per q-chunk: qc1 folds on GPSIMD so the three
        # chaser chains don't serialize on the DVE at stream end.
        def _fold_eng(qc):
            return nc.gpsimd if qc == 1 else nc.vector

        def emit_den(qc, pr, tail_mode):
            """Denominator pieces as A^T tiles become available. DVE adds are
            kept to [P,4,QC] so they can't head-of-line-block V drains long.
            Tree mode: all 32 tiles accumulated + folded + AR.
            Tail mode: tiles 0..23 only (folded at pr11); tiles 24..31 go
            through PE ones-matmuls so the post-last-exp latency is tiny."""
            a = a_t[qc]
            if pr == 7:
                t1a[qc] = dpool.tile([P, 4, QC], BF16, name="t1a", bufs=3)
                t1b[qc] = dpool.tile([P, 4, QC], BF16, name="t1b", bufs=3)
                nc.vector.tensor_add(t1a[qc][:], a[:, 0:4, :], a[:, 4:8, :])
                nc.vector.tensor_add(t1b[qc][:], a[:, 8:12, :], a[:, 12:16, :])
            elif pr == 11:
                nc.vector.tensor_add(t1a[qc][:], t1a[qc][:], a[:, 16:20, :])
                nc.vector.tensor_add(t1b[qc][:], t1b[qc][:], a[:, 20:24, :])
                if tail_mode:
                    _fold_and_reduce(qc)
            elif pr == 13 and not tail_mode:
                nc.vector.tensor_add(t1a[qc][:], t1a[qc][:], a[:, 24:28, :])
            elif pr == 15 and not tail_mode:
                nc.vector.tensor_add(t1b[qc][:], t1b[qc][:], a[:, 28:32, :])
                _fold_and_reduce(qc)

        def _fold_and_reduce(qc):
            eng = _fold_eng(qc)
            f1 = dpool.tile([P, 4, QC], BF16, name="f1")
            eng.tensor_add(f1[:], t1a[qc][:], t1b[qc][:])
            eng.tensor_add(f1[:, 0:2, :], f1[:, 0:2, :], f1[:, 2:4, :])
            den128 = dpool.tile([P, QC], F32, name="den128")
            eng.tensor_add(den128[:], f1[:, 0, :], f1[:, 1, :])
            dall = dpool.tile([P, QC], F32, name="den_all")
            nc.gpsimd.partition_all_reduce(dall[:], den128[:], P,
                                           bass.bass_isa.ReduceOp.add)
            den_all[qc] = dall

        def emit_drain(qc):
            """O^T psum -> bf16 SBUF; frees the opsum slot."""
            oT = otpool.tile([P, QC], BF16, name="oT")
            nc.vector.tensor_copy(oT[:], o_ps[qc][:])
            return oT

        oT_sb = {}

        def emit_finalize(qc, fin, tail_mode):
            """Per q-tile: denominator transpose + reciprocal, O^T transpose,
            scale, and the output DMA."""
            if tail_mode:
                tail_sb = dpool.tile([1, QC], F32, name="tail_sb", bufs=1)
                nc.vector.tensor_copy(tail_sb[:], tail[qc][:])
                dcomb = dpool.tile([1, QC], F32, name="dcomb", bufs=1)
                nc.vector.tensor_add(dcomb[:], den_all[qc][:1, :], tail_sb[:])
                dsrc = dcomb
            else:
                dsrc = den_all[qc]
            oT = oT_sb[qc]
            for t in range(QT):
                ps_dt = fin.tile([P, 1], F32, name="fin_t")
                nc.tensor.transpose(ps_dt[:], dsrc[:1, bass.ts(t, P)],
                                    ident[:1, :1])
                rden = dpool.tile([P, 1], F32, name="rden", bufs=2 * QT)
                nc.vector.reciprocal(rden[:], ps_dt[:])
                ps_ot = fin.tile([P, D], BF16, name="fin_t")
                nc.tensor.transpose(ps_ot[:], oT[:, bass.ts(t, P)], identb[:])
                nc.vector.tensor_scalar_mul(o_sb[:, qc * QT + t, :],
                                            ps_ot[:], rden[:])
            nc.sync.dma_start(out_r[:, qc * QT:(qc + 1) * QT, :],
                              o_sb[:, qc * QT:(qc + 1) * QT, :])

        # PE warm-up: ~3.3us of back-to-back transposes ramps the PE to its
        # full 2.4 GHz p-state before the first real matmul arrives.
        with tc.tile_pool(name="warm", bufs=1, space="PSUM") as wp:
            warm_ps = wp.tile([P, P], BF16, name="warm_ps")
            for _ in range(34):
                nc.tensor.transpose(warm_ps[:], identb[:], identb[:])

        # ---- streaming phase: projections + chasing q-chunks 0..NCH-1 ----
        pend = [0] * NCH
        with tc.tile_pool(name="pp", bufs=2, space="PSUM") as pp:
            for sc in range(NSC):
                ssl = bass.ds(sc * SC, SC)
                if sc == 0:
                    x_r = x_r0
                else:
                    x_r = xstage.tile([P, MT, SC], BF16, name="x_r")
                    nc.sync.dma_start(x_r[:], xT_r[:, :, ssl])

                # K^T chunk
                ps = pp.tile([P, SC], F32, name="pp")
                for mt in range(MT):
                    nc.tensor.matmul(ps[:], wk_sb[:, mt, :], x_r[:, mt, :],
                                     start=(mt == 0), stop=(mt == MT - 1))
                nc.scalar.activation(kT_sb[:, ssl], ps[:], Ident, bias=bk_sb)

                # Q^T chunk (query rows are the permuted prefix)
                if sc * SC < S_q:
                    ps2 = pp.tile([P, SC], F32, name="pp")
                    for mt in range(MT):
                        nc.tensor.matmul(ps2[:], wq_sb[:, mt, :], x_r[:, mt, :],
                                         start=(mt == 0), stop=(mt == MT - 1))
                    nc.scalar.activation(qT_sb[:, ssl], ps2[:], Ident,
                                         bias=bq_sb)

                # V chunk in natural [s, dv] layout: x-tile stationary
                for t in range(SCT):
                    st = sc * SCT + t
                    psv = pp.tile([P, D], F32, name="pp")
                    for mt in range(MT):
                        nc.tensor.matmul(psv[:], x_r[:, mt, bass.ts(t, P)],
                                         wv_sb[:, mt, :],
                                         start=(mt == 0), stop=(mt == MT - 1))
                    nc.vector.tensor_add(v_sb[:, st, :], psv[:], bv_bcast[:])

                # chasing q-chunks catch up on all available pairs; q-chunks
                # 0..FULL_CH-1 also run their attn@V in-stream (they own the
                # two in-stream O^T psum banks); further chasers defer attn@V
                # to the post-stream phase.
                avail = (sc + 1) * SCT // 2
                for qc in range(NCH):
                    if (qc + 1) * QC <= (sc + 1) * SC:
                        while pend[qc] < avail:
                            pr = pend[qc]
                            emit_scores(qc, pr)
                            if qc < FULL_CH:
                                emit_av(qc, pr)
                            emit_den(qc, pr, tail_mode=False)
                            pend[qc] += 1

        # ---- post-stream: drains, qc2's deferred attn@V, qc3, finalize ----
        # qc3's scores/exps start immediately (ACT is its critical path);
        # qc2's deferred attn@V (2 pairs per iteration) fills the PE slack.
        # Chaser finalizes are injected once their den chains complete; qc3
        # uses the tail-mode den so its own finalize is prompt.
        with tc.tile_pool(name="fin", bufs=2, space="PSUM") as fin:
            for qc in range(FULL_CH):
                oT_sb[qc] = emit_drain(qc)

            lq = NQC - 1
            fin_sched = {5: 0, 9: 1, 12: 2}  # qc3 iteration -> finalize qc
            LAG = 2
            for pr in range(NPAIR):
                emit_scores(lq, pr)
                if NCH > FULL_CH and pr < NPAIR // 2:
                    # qc2 deferred attn@V: 2 pairs per iteration
                    emit_av(2, 2 * pr)
                    emit_av(2, 2 * pr + 1)
                    if pr == NPAIR // 2 - 1:
                        oT_sb[2] = emit_drain(2)
                if pr >= LAG:
                    emit_av(lq, pr - LAG, tail_mode=True)
                    emit_den(lq, pr - LAG, tail_mode=True)
                fqc = fin_sched.get(pr)
                if fqc is not None and fqc < lq:
                    emit_finalize(fqc, fin, tail_mode=False)
            for pr in range(NPAIR - LAG, NPAIR):
                emit_av(lq, pr, tail_mode=True)
                emit_den(lq, pr, tail_mode=True)

            oT_sb[lq] = emit_drain(lq)
            emit_finalize(lq, fin, tail_mode=True)
        ctx.close()

    return nc


def build(n_cores=N_CORES, **kw):
    nc = bacc.Bacc("TRN2", target_bir_lowering=False, debug=False,
                   num_devices=n_cores)
    build_attention(nc, **kw)
    nc.compile()
    return nc


def shard_inputs(input, Wq, bq, Wk, bk, Wv, bv):
    """Per-core in_maps. Core c: batch c//2, query-half c%2; the host permutes
    the batch so the core's query rows come first (softmax is key-permutation
    invariant), transposes to xT [M, S], and converts to bf16."""
    import ml_dtypes
    half = S // 2
    MT = M // 128

    def pack_w(W):
        # [M, D] -> [P, MT*D] with [p, mt*D + d] = W[mt*128 + p, d]
        return np.ascontiguousarray(
            np.asarray(W, dtype=np.float32).reshape(MT, 128, D)
            .transpose(1, 0, 2).reshape(128, MT * D)).astype(ml_dtypes.bfloat16)

    wq_b, wk_b, wv_b = pack_w(Wq), pack_w(Wk), pack_w(Wv)
    bkq_f = np.ascontiguousarray(np.stack(
        [np.asarray(bk, dtype=np.float32).ravel(),
         np.asarray(bq, dtype=np.float32).ravel()], axis=1))
    bv_f = np.asarray(bv, dtype=np.float32).reshape(1, D)
    in_maps = []
    for c in range(N_CORES):
        b, h = divmod(c, 2)
        xb = np.asarray(input[b])
        x_perm = np.concatenate(
            [xb[h * half:(h + 1) * half], xb[(1 - h) * half:(2 - h) * half]],
            axis=0)
        xT = np.ascontiguousarray(x_perm.T).astype(ml_dtypes.bfloat16)
        in_maps.append({
            "xT": xT,
            "wq": wq_b, "wk": wk_b, "wv": wv_b,
            "bkq": bkq_f, "bv": bv_f,
        })
    return in_maps


_NC_CACHE = {}


def kernel(input, Wq, bq, Wk, bk, Wv, bv):
    in_maps = shard_inputs(input, Wq, bq, Wk, bk, Wv, bv)
    if "nc" not in _NC_CACHE:
        _NC_CACHE["nc"] = build()
    nc = _NC_CACHE["nc"]
    res = run_bass_kernel_spmd(nc, in_maps, core_ids=list(range(N_CORES)))
    half = S // 2
    result = np.empty((B, S, D), dtype=np.float32)
    for c in range(N_CORES):
        b, h = divmod(c, 2)
        result[b, h * half:(h + 1) * half] = np.asarray(
            res.results[c]["out"]).astype(np.float32)
    return result


if __name__ == "__main__":
    rng = np.random.default_rng(0)
    inputs = {
        "input": rng.standard_normal((B, S, M), dtype=np.float32),
        "Wq": (rng.standard_normal((M, D), dtype=np.float32) / np.sqrt(M)).astype(np.float32),
        "bq": (rng.standard_normal(D, dtype=np.float32) * 0.02),
        "Wk": (rng.standard_normal((M, D), dtype=np.float32) / np.sqrt(M)).astype(np.float32),
        "bk": (rng.standard_normal(D, dtype=np.float32) * 0.02),
        "Wv": (rng.standard_normal((M, D), dtype=np.float32) / np.sqrt(M)).astype(np.float32),
        "bv": (rng.standard_normal(D, dtype=np.float32) * 0.02),
    }
    out = kernel(**inputs)
    print("kernel output:", out.shape, out.dtype)


# revision 41
# speedup vs baseline: 1.2947x; 1.0028x over previous
"""Self-contained Trainium2 Bass kernel for a single attention head.

Reference computation (per batch b):
    Q = x @ Wq + bq ; K = x @ Wk + bk ; V = x @ Wv + bv      (x: [S, M])
    out = softmax(Q K^T / sqrt(D)) @ V                        ([S, D])

Shapes: B=4, S=4096, M=1024, D=128, f32.

Sharding: 8 cores; core c handles batch b=c//2, query-half h=c%2 (2048 query
rows), with the full batch (4096 rows) as keys/values. Softmax is over the
key axis only, so key order is irrelevant: the host permutes each core's
batch so its own query rows come first, pre-transposes to xT [M, S], and
converts to bf16 (halves input DMA; rel-err budget measured ~4e-3).

Device pipeline (fully fused, no phase barrier):
  - stream 8 s-chunks (512 keys): K^T/Q^T chunks via bf16 matmuls (moving
    dim 512) + ACT bias-drains; V in natural [s, dv] layout directly via
    x-tile-stationary matmuls (no transposes) + DVE bias-drains.
  - q-chunks 0/1 chase the stream: scores S^T[s,q] (bf16, PSUM), one wide
    exp per pair [128,1024] -> A^T bf16 (exactly the attn@V layout), attn@V
    accumulates O^T in PSUM across all 32 s-tiles as they appear.
  - q-chunks 2/3 run after the stream from resident K^T/V.
  - softmax denominator: DVE bf16 wide adds (2x mode) -> f32 fold ->
    GPSIMD partition_all_reduce; for the last q-chunk the final 8 s-tiles
    accumulate on the PE via ones-matmuls so the post-exp tail is short.
  - finalize per qc: O^T -> bf16 -> PE transpose -> DVE scale by 1/den ->
    bf16 out DMA (host converts to f32).
  - PSUM budget 8 banks: proj pool 2 + scores 4 + O^T/tail 2, finalize
    pool reuses the closed proj pool's banks.
"""

from contextlib import ExitStack

import numpy as np

import concourse.bass as bass
import concourse.tile as tile
from concourse import bacc, mybir
from concourse.bass_utils import run_bass_kernel_spmd
from concourse.masks import make_identity

F32 = mybir.dt.float32
BF16 = mybir.dt.bfloat16

B, S, M, D = 4, 4096, 1024, 128
N_CORES = 8
P = 128
SCALE = 1.0 / np.sqrt(np.float32(D))


def build_attention(nc, S_keys=S, S_q=S // 2, M_dim=M, SC=512, QC=512):
    MT = M_dim // P               # m-tiles (8)
    ST = S_keys // P              # key s-tiles (32)
    NSC = S_keys // SC            # s-chunks (8)
    SCT = SC // P                 # s-tiles per chunk (4)
    NQC = S_q // QC               # q-chunks (4)
    QT = QC // P                  # q-tiles per q-chunk (4)
    NCH = min(3, NQC - 1)         # q-chunks chasing the projection stream
    FULL_CH = min(2, NCH)         # chasers that also run attn@V in-stream
    NPAIR = ST // 2               # score pairs per q-chunk (16)

    # weights host-packed [P, MT*D] (contiguous per partition), biases packed
    # into one [P, 2] (bk|bq) + one [1, D] (bv) tensor so their DMAs are tiny
    # and early.
    xT = nc.dram_tensor("xT", [M_dim, S_keys], BF16, kind="ExternalInput").ap()
    wq = nc.dram_tensor("wq", [P, MT * D], BF16, kind="ExternalInput").ap()
    wk = nc.dram_tensor("wk", [P, MT * D], BF16, kind="ExternalInput").ap()
    wv = nc.dram_tensor("wv", [P, MT * D], BF16, kind="ExternalInput").ap()
    bkq = nc.dram_tensor("bkq", [P, 2], F32, kind="ExternalInput").ap()
    bv = nc.dram_tensor("bv", [1, D], F32, kind="ExternalInput").ap()
    out = nc.dram_tensor("out", [S_q, D], BF16, kind="ExternalOutput").ap()

    xT_r = xT.rearrange("(t p) s -> p t s", p=P)
    out_r = out.rearrange("(t p) d -> p t d", p=P)

    Ident = mybir.ActivationFunctionType.Identity
    Exp = mybir.ActivationFunctionType.Exp

    with tile.TileContext(nc) as tc:
        ctx = ExitStack()
        persist = ctx.enter_context(tc.tile_pool(name="persist", bufs=1))

        ident = persist.tile([P, P], F32)
        make_identity(nc, ident[:])
        identb = persist.tile([P, P], BF16)
        nc.vector.tensor_copy(identb[:], ident[:])
        ones_f = persist.tile([P, 1], F32)
        nc.vector.memset(ones_f[:], 1.0)
        onesb = persist.tile([P, 1], BF16)
        nc.vector.tensor_copy(onesb[:], ones_f[:])

        # DMA issue order matters at startup (the engine serializes transfers
        # in issue order; the gpsimd queue dispatches fastest): wk, then x
        # chunk 0 in halves, then wq/wv (chunk-0 Q/V projections), then the
        # chunk-1 prefetch, then biases.
        xstage = ctx.enter_context(tc.tile_pool(name="xstage", bufs=2))
        wk_sb = persist.tile([P, MT, D], BF16)
        nc.gpsimd.dma_start(wk_sb[:], wk.rearrange("p (t d) -> p t d", d=D))
        bkq_sb = persist.tile([P, 2], F32)
        nc.sync.dma_start(bkq_sb[:], bkq)
        bk_sb = bkq_sb[:, 0:1]
        bq_sb = bkq_sb[:, 1:2]
        bv_row = persist.tile([1, D], F32)
        nc.sync.dma_start(bv_row[:], bv)
        bv_bcast = persist.tile([P, D], F32)
        nc.gpsimd.partition_broadcast(bv_bcast[:], bv_row[:])
        x_r0 = xstage.tile([P, MT, SC], BF16, name="x_r")
        nc.sync.dma_start(x_r0[:, 0:MT // 2, :],
                          xT_r[:, 0:MT // 2, bass.ds(0, SC)])
        nc.sync.dma_start(x_r0[:, MT // 2:, :],
                          xT_r[:, MT // 2:, bass.ds(0, SC)])
        wq_sb = persist.tile([P, MT, D], BF16)
        nc.sync.dma_start(wq_sb[:], wq.rearrange("p (t d) -> p t d", d=D))
        wv_sb = persist.tile([P, MT, D], BF16)
        nc.sync.dma_start(wv_sb[:], wv.rearrange("p (t d) -> p t d", d=D))

        kT_sb = persist.tile([P, S_keys], BF16)    # K^T  [dk, s]
        qT_sb = persist.tile([P, S_q], BF16)       # Q^T  [dk, q]
        v_sb = persist.tile([P, ST, D], BF16)      # V    [s%128, s-tile, dv]
        o_sb = persist.tile([P, S_q // P, D], BF16)  # O   [q%128, q-tile, dv]

        apool = ctx.enter_context(tc.tile_pool(name="apool", bufs=3))
        dpool = ctx.enter_context(tc.tile_pool(name="dpool", bufs=2))
        otpool = ctx.enter_context(tc.tile_pool(name="otpool", bufs=2))
        spsum = ctx.enter_context(tc.tile_pool(name="spsum", bufs=2, space="PSUM"))
        opsum = ctx.enter_context(tc.tile_pool(name="opsum", bufs=2, space="PSUM"))

        a_t = {}      # qc -> A^T tile [P, ST, QC] bf16
        o_ps = {}     # qc -> O^T psum [P, QC]
        t1a = {}      # qc -> den partials [P, 4, QC] bf16 (tiles 0,8,16.. lanes)
        t1b = {}
        den_all = {}  # qc -> all-reduced denominator [P, QC] f32
        tail = {}     # tail-mode PE-side den psum [1, QC]

        def qsl(qc):
            return bass.ds(qc * QC, QC)

        def emit_scores(qc, pr, split_exp=False):
            """Scores for s-tiles (2pr, 2pr+1) x q-chunk qc + one wide exp.
            split_exp emits two narrow exps instead (shortens the dependence
            tail of the very last pair)."""
            if pr == 0:
                a_t[qc] = apool.tile([P, ST, QC], BF16, name="a_sb")
            ps_s = spsum.tile([P, 2, QC], F32, name="ps_s")
            for j in range(2):
                st = 2 * pr + j
                nc.tensor.matmul(ps_s[:, j, :], kT_sb[:, bass.ts(st, P)],
                                 qT_sb[:, qsl(qc)], start=True, stop=True)
            if split_exp:
                for j in range(2):
                    st = 2 * pr + j
                    nc.scalar.activation(a_t[qc][:, st:st + 1, :],
                                         ps_s[:, j:j + 1, :], Exp,
                                         scale=float(SCALE))
            else:
                nc.scalar.activation(a_t[qc][:, 2 * pr:2 * pr + 2, :], ps_s[:],
                                     Exp, scale=float(SCALE))

        def emit_av(qc, pr, tail_mode=False):
            """attn@V accumulation matmuls for pair pr (+ the PE ones-matmul
            denominator tail in tail mode)."""
            if qc not in o_ps:
                o_ps[qc] = opsum.tile([P, QC], F32, name="o_ps")
            for j in range(2):
                st = 2 * pr + j
                nc.tensor.matmul(o_ps[qc][:], v_sb[:, st, :],
                                 a_t[qc][:, st, :],
                                 start=(st == 0), stop=(st == ST - 1))
            if tail_mode and pr >= 12:
                if pr == 12:
                    tail[qc] = opsum.tile([1, QC], F32, name="o_ps")
                for j in range(2):
                    st = 2 * pr + j
                    nc.tensor.matmul(tail[qc][:], onesb[:], a_t[qc][:, st, :],
                                     start=(st == 24), stop=(st == ST - 1))

        # den fold engine per q-chunk: qc1 folds on GPSIMD so the three
        # chaser chains don't serialize on the DVE at stream end.
        def _fold_eng(qc):
            return nc.gpsimd if qc == 1 else nc.vector

        def emit_den(qc, pr, tail_mode):
            """Denominator pieces as A^T tiles become available. DVE adds are
            kept to [P,4,QC] so they can't head-of-line-block V drains long.
            Tree mode: all 32 tiles accumulated + folded + AR.
            Tail mode: tiles 0..23 only (folded at pr11); tiles 24..31 go
            through PE ones-matmuls so the post-last-exp latency is tiny."""
            a = a_t[qc]
            if pr == 7:
                t1a[qc] = dpool.tile([P, 4, QC], BF16, name="t1a", bufs=3)
                t1b[qc] = dpool.tile([P, 4, QC], BF16, name="t1b", bufs=3)
                nc.vector.tensor_add(t1a[qc][:], a[:, 0:4, :], a[:, 4:8, :])
                nc.vector.tensor_add(t1b[qc][:], a[:, 8:12, :], a[:, 12:16, :])
            elif pr == 11:
                nc.vector.tensor_add(t1a[qc][:], t1a[qc][:], a[:, 16:20, :])
                nc.vector.tensor_add(t1b[qc][:], t1b[qc][:], a[:, 20:24, :])
                if tail_mode:
                    _fold_and_reduce(qc)
            elif pr == 13 and not tail_mode:
                nc.vector.tensor_add(t1a[qc][:], t1a[qc][:], a[:, 24:28, :])
            elif pr == 15 and not tail_mode:
                nc.vector.tensor_add(t1b[qc][:], t1b[qc][:], a[:, 28:32, :])
                _fold_and_reduce(qc)

        def _fold_and_reduce(qc):
            eng = _fold_eng(qc)
            f1 = dpool.tile([P, 4, QC], BF16, name="f1")
            eng.tensor_add(f1[:], t1a[qc][:], t1b[qc][:])
            eng.tensor_add(f1[:, 0:2, :], f1[:, 0:2, :], f1[:, 2:4, :])
            den128 = dpool.tile([P, QC], F32, name="den128")
            eng.tensor_add(den128[:], f1[:, 0, :], f1[:, 1, :])
            dall = dpool.tile([P, QC], F32, name="den_all")
            nc.gpsimd.partition_all_reduce(dall[:], den128[:], P,
                                           bass.bass_isa.ReduceOp.add)
            den_all[qc] = dall

        def emit_drain(qc, on_act=False):
            """O^T psum -> bf16 SBUF; frees the opsum slot. The last drain
            goes on the ACT engine (idle at the end) so it runs concurrently
            with the DVE den-tail work."""
            oT = otpool.tile([P, QC], BF16, name="oT")
            if on_act:
                nc.scalar.copy(oT[:], o_ps[qc][:])
            else:
                nc.vector.tensor_copy(oT[:], o_ps[qc][:])
            return oT

        oT_sb = {}

        def emit_finalize(qc, fin, tail_mode):
            """Per q-tile: denominator transpose + reciprocal, O^T transpose,
            scale, and the output DMA."""
            if tail_mode:
                # tail_sb copy on ACT so it runs concurrently with the DVE
                # O^T drain; den transposes + reciprocals are emitted before
                # the O^T transposes to shorten the serial sem chain.
                tail_sb = dpool.tile([1, QC], F32, name="tail_sb", bufs=1)
                nc.scalar.copy(tail_sb[:], tail[qc][:])
                dcomb = dpool.tile([1, QC], F32, name="dcomb", bufs=1)
                nc.vector.tensor_add(dcomb[:], den_all[qc][:1, :], tail_sb[:])
                dsrc = dcomb
            else:
                dsrc = den_all[qc]
            oT = oT_sb[qc]
            rdens = []
            for t in range(QT):
                ps_dt = fin.tile([P, 1], F32, name="fin_t")
                nc.tensor.transpose(ps_dt[:], dsrc[:1, bass.ts(t, P)],
                                    ident[:1, :1])
                rden = dpool.tile([P, 1], F32, name="rden", bufs=2 * QT)
                nc.vector.reciprocal(rden[:], ps_dt[:])
                rdens.append(rden)
            for t in range(QT):
                ps_ot = fin.tile([P, D], BF16, name="fin_t")
                nc.tensor.transpose(ps_ot[:], oT[:, bass.ts(t, P)], identb[:])
                nc.vector.tensor_scalar_mul(o_sb[:, qc * QT + t, :],
                                            ps_ot[:], rdens[t][:])
                if tail_mode and t % 2 == 1:
                    nc.sync.dma_start(out_r[:, qc * QT + t - 1:qc * QT + t + 1, :],
                                      o_sb[:, qc * QT + t - 1:qc * QT + t + 1, :])
            if not tail_mode:
                nc.sync.dma_start(out_r[:, qc * QT:(qc + 1) * QT, :],
                                  o_sb[:, qc * QT:(qc + 1) * QT, :])

        # PE warm-up: ~3.3us of back-to-back transposes ramps the PE to its
        # full 2.4 GHz p-state before the first real matmul arrives.
        with tc.tile_pool(name="warm", bufs=1, space="PSUM") as wp:
            warm_ps = wp.tile([P, P], BF16, name="warm_ps")
            for _ in range(34):
                nc.tensor.transpose(warm_ps[:], identb[:], identb[:])

        # ---- streaming phase: projections + chasing q-chunks 0..NCH-1 ----
        pend = [0] * NCH
        with tc.tile_pool(name="pp", bufs=2, space="PSUM") as pp:
            for sc in range(NSC):
                ssl = bass.ds(sc * SC, SC)
                if sc == 0:
                    x_r = x_r0
                else:
                    x_r = xstage.tile([P, MT, SC], BF16, name="x_r")
                    nc.sync.dma_start(x_r[:], xT_r[:, :, ssl])

                # K^T chunk
                ps = pp.tile([P, SC], F32, name="pp")
                for mt in range(MT):
                    nc.tensor.matmul(ps[:], wk_sb[:, mt, :], x_r[:, mt, :],
                                     start=(mt == 0), stop=(mt == MT - 1))
                nc.scalar.activation(kT_sb[:, ssl], ps[:], Ident, bias=bk_sb)

                # Q^T chunk (query rows are the permuted prefix)
                if sc * SC < S_q:
                    ps2 = pp.tile([P, SC], F32, name="pp")
                    for mt in range(MT):
                        nc.tensor.matmul(ps2[:], wq_sb[:, mt, :], x_r[:, mt, :],
                                         start=(mt == 0), stop=(mt == MT - 1))
                    nc.scalar.activation(qT_sb[:, ssl], ps2[:], Ident,
                                         bias=bq_sb)

                # V chunk in natural [s, dv] layout: x-tile stationary
                for t in range(SCT):
                    st = sc * SCT + t
                    psv = pp.tile([P, D], F32, name="pp")
                    for mt in range(MT):
                        nc.tensor.matmul(psv[:], x_r[:, mt, bass.ts(t, P)],
                                         wv_sb[:, mt, :],
                                         start=(mt == 0), stop=(mt == MT - 1))
                    nc.vector.tensor_add(v_sb[:, st, :], psv[:], bv_bcast[:])

                # chasing q-chunks catch up on all available pairs; q-chunks
                # 0..FULL_CH-1 also run their attn@V in-stream (they own the
                # two in-stream O^T psum banks); further chasers defer attn@V
                # to the post-stream phase.
                avail = (sc + 1) * SCT // 2
                for qc in range(NCH):
                    if (qc + 1) * QC <= (sc + 1) * SC:
                        while pend[qc] < avail:
                            pr = pend[qc]
                            emit_scores(qc, pr)
                            if qc < FULL_CH:
                                emit_av(qc, pr)
                            emit_den(qc, pr, tail_mode=False)
                            pend[qc] += 1

        # ---- post-stream: drains, qc2's deferred attn@V, qc3, finalize ----
        # qc3's scores/exps start immediately (ACT is its critical path);
        # qc2's deferred attn@V (2 pairs per iteration) fills the PE slack.
        # Chaser finalizes are injected once their den chains complete; qc3
        # uses the tail-mode den so its own finalize is prompt.
        with tc.tile_pool(name="fin", bufs=2, space="PSUM") as fin:
            for qc in range(FULL_CH):
                oT_sb[qc] = emit_drain(qc)

            lq = NQC - 1
            fin_sched = {5: 0, 9: 1, 12: 2}  # qc3 iteration -> finalize qc
            LAG = 2
            for pr in range(NPAIR):
                emit_scores(lq, pr, split_exp=(pr == NPAIR - 1))
                if NCH > FULL_CH and pr < NPAIR // 2:
                    # qc2 deferred attn@V: 2 pairs per iteration
                    emit_av(2, 2 * pr)
                    emit_av(2, 2 * pr + 1)
                    if pr == NPAIR // 2 - 1:
                        oT_sb[2] = emit_drain(2)
                if pr >= LAG:
                    emit_av(lq, pr - LAG, tail_mode=True)
                    emit_den(lq, pr - LAG, tail_mode=True)
                fqc = fin_sched.get(pr)
                if fqc is not None and fqc < lq:
                    emit_finalize(fqc, fin, tail_mode=False)
            for pr in range(NPAIR - LAG, NPAIR):
                emit_av(lq, pr, tail_mode=True)
                emit_den(lq, pr, tail_mode=True)

            oT_sb[lq] = emit_drain(lq, on_act=True)
            emit_finalize(lq, fin, tail_mode=True)
        ctx.close()

    return nc


def build(n_cores=N_CORES, **kw):
    nc = bacc.Bacc("TRN2", target_bir_lowering=False, debug=False,
                   num_devices=n_cores)
    build_attention(nc, **kw)
    nc.compile()
    return nc


def shard_inputs(input, Wq, bq, Wk, bk, Wv, bv):
    """Per-core in_maps. Core c: batch c//2, query-half c%2; the host permutes
    the batch so the core's query rows come first (softmax is key-permutation
    invariant), transposes to xT [M, S], and converts to bf16."""
    import ml_dtypes
    half = S // 2
    MT = M // 128

    def pack_w(W):
        # [M, D] -> [P, MT*D] with [p, mt*D + d] = W[mt*128 + p, d]
        return np.ascontiguousarray(
            np.asarray(W, dtype=np.float32).reshape(MT, 128, D)
            .transpose(1, 0, 2).reshape(128, MT * D)).astype(ml_dtypes.bfloat16)

    wq_b, wk_b, wv_b = pack_w(Wq), pack_w(Wk), pack_w(Wv)
    bkq_f = np.ascontiguousarray(np.stack(
        [np.asarray(bk, dtype=np.float32).ravel(),
         np.asarray(bq, dtype=np.float32).ravel()], axis=1))
    bv_f = np.asarray(bv, dtype=np.float32).reshape(1, D)
    in_maps = []
    for c in range(N_CORES):
        b, h = divmod(c, 2)
        xb = np.asarray(input[b])
        x_perm = np.concatenate(
            [xb[h * half:(h + 1) * half], xb[(1 - h) * half:(2 - h) * half]],
            axis=0)
        xT = np.ascontiguousarray(x_perm.T).astype(ml_dtypes.bfloat16)
        in_maps.append({
            "xT": xT,
            "wq": wq_b, "wk": wk_b, "wv": wv_b,
            "bkq": bkq_f, "bv": bv_f,
        })
    return in_maps


_NC_CACHE = {}


def kernel(input, Wq, bq, Wk, bk, Wv, bv):
    in_maps = shard_inputs(input, Wq, bq, Wk, bk, Wv, bv)
    if "nc" not in _NC_CACHE:
        _NC_CACHE["nc"] = build()
    nc = _NC_CACHE["nc"]
    res = run_bass_kernel_spmd(nc, in_maps, core_ids=list(range(N_CORES)))
    half = S // 2
    result = np.empty((B, S, D), dtype=np.float32)
    for c in range(N_CORES):
        b, h = divmod(c, 2)
        result[b, h * half:(h + 1) * half] = np.asarray(
            res.results[c]["out"]).astype(np.float32)
    return result


if __name__ == "__main__":
    rng = np.random.default_rng(0)
    inputs = {
        "input": rng.standard_normal((B, S, M), dtype=np.float32),
        "Wq": (rng.standard_normal((M, D), dtype=np.float32) / np.sqrt(M)).astype(np.float32),
        "bq": (rng.standard_normal(D, dtype=np.float32) * 0.02),
        "Wk": (rng.standard_normal((M, D), dtype=np.float32) / np.sqrt(M)).astype(np.float32),
        "bk": (rng.standard_normal(D, dtype=np.float32) * 0.02),
        "Wv": (rng.standard_normal((M, D), dtype=np.float32) / np.sqrt(M)).astype(np.float32),
        "bv": (rng.standard_normal(D, dtype=np.float32) * 0.02),
    }
    out = kernel(**inputs)
    print("kernel output:", out.shape, out.dtype)


# revision 43
# speedup vs baseline: 1.4050x; 1.0852x over previous
"""Self-contained Trainium2 Bass kernel for a single attention head.

Reference computation (per batch b):
    Q = x @ Wq + bq ; K = x @ Wk + bk ; V = x @ Wv + bv      (x: [S, M])
    out = softmax(Q K^T / sqrt(D)) @ V                        ([S, D])

Shapes: B=4, S=4096, M=1024, D=128, f32.

Sharding (key-split + host merge): 8 cores; core c handles batch b=c//2 and
KEY-half h=c%2. Each core projects K/V for its own 2048 key rows only, Q for
all 4096 queries, and computes the UNNORMALIZED partial attention
  N_h^T[dv, q] = sum_{s in half h} exp(q.k_s/sqrt(D)) v_s,   d_h[q] = sum_s exp(.)
over its key half. The host merges: O = (N_0 + N_1) / (d_0 + d_1) — softmax
over the key axis is an exact sum-decomposition, so the merge is exact. This
halves the redundant K/V projection work and removes every on-device
normalization/transpose step (the host divides and transposes). The host
permutes each core's rows so its key half comes first (key order inside a
softmax is irrelevant; the query order is un-permuted on the host).

Device pipeline (fully fused stream over 8 x-chunks of 512 rows):
  - chunks 0-3 project K^T/V (own keys) + Q^T; chunks 4-7 project Q^T only.
    x is bf16 (host-converted; halves DMA). V is built in natural [s, dv]
    layout directly via x-tile-stationary matmuls (no transposes).
  - 8 q-chunks chase the stream: scores S^T[s,q] (bf16 matmuls into PSUM),
    one wide exp per s-tile pair [128,1024] -> A^T bf16, attn@V accumulates
    N^T in PSUM. Two q-chunks hold the two O-psum banks at a time; the
    others defer attn@V until a slot frees (their A^T stays in SBUF).
  - denominator: per q-chunk DVE bf16 tree -> f32 -> GPSIMD
    partition_all_reduce -> DMA; the last q-chunk accumulates its den with
    PE ones-matmuls instead so the post-last-exp tail is tiny.
  - N^T q-slabs DMA out straight from the PSUM drain; no finalize pass.
  - PSUM (8 banks): scores 2x2 + N^T accumulators 2 + projections 2.
"""

from contextlib import ExitStack

import numpy as np

import concourse.bass as bass
import concourse.tile as tile
from concourse import bacc, mybir
from concourse.bass_utils import run_bass_kernel_spmd
from concourse.masks import make_identity

F32 = mybir.dt.float32
BF16 = mybir.dt.bfloat16

B, S, M, D = 4, 4096, 1024, 128
N_CORES = 8
P = 128
SCALE = 1.0 / np.sqrt(np.float32(D))


def build_attention(nc, S_all=S, M_dim=M, SC=512, QC=512):
    KH = S_all // 2               # keys per core (2048)
    MT = M_dim // P               # m-tiles (8)
    ST = KH // P                  # key s-tiles (16)
    NSC = S_all // SC             # x-chunks (8)
    NKC = KH // SC                # key chunks (4)
    SCT = SC // P                 # s-tiles per chunk (4)
    NQC = S_all // QC             # q-chunks (8)
    NPAIR = ST // 2               # score pairs per q-chunk (8)
    LQ = NQC - 1                  # last q-chunk (PE-ones denominator)

    xT = nc.dram_tensor("xT", [M_dim, S_all], BF16, kind="ExternalInput").ap()
    wq = nc.dram_tensor("wq", [P, MT * D], BF16, kind="ExternalInput").ap()
    wk = nc.dram_tensor("wk", [P, MT * D], BF16, kind="ExternalInput").ap()
    wv = nc.dram_tensor("wv", [P, MT * D], BF16, kind="ExternalInput").ap()
    bkq = nc.dram_tensor("bkq", [P, 2], F32, kind="ExternalInput").ap()
    bv = nc.dram_tensor("bv", [1, D], F32, kind="ExternalInput").ap()
    out = nc.dram_tensor("out", [D, S_all], BF16, kind="ExternalOutput").ap()
    den = nc.dram_tensor("den", [1, S_all], F32, kind="ExternalOutput").ap()

    xT_r = xT.rearrange("(t p) s -> p t s", p=P)

    Ident = mybir.ActivationFunctionType.Identity
    Exp = mybir.ActivationFunctionType.Exp

    with tile.TileContext(nc) as tc:
        ctx = ExitStack()
        persist = ctx.enter_context(tc.tile_pool(name="persist", bufs=1))

        ident = persist.tile([P, P], F32)
        make_identity(nc, ident[:])
        identb = persist.tile([P, P], BF16)
        nc.vector.tensor_copy(identb[:], ident[:])
        ones_f = persist.tile([P, 1], F32)
        nc.vector.memset(ones_f[:], 1.0)
        onesb = persist.tile([P, 1], BF16)
        nc.vector.tensor_copy(onesb[:], ones_f[:])

        # startup DMA order: wk (gpsimd queue, instant issue), then biases +
        # x chunk 0 halves + wq/wv on the SP queue in priority order
        xstage = ctx.enter_context(tc.tile_pool(name="xstage", bufs=2))
        wk_sb = persist.tile([P, MT, D], BF16)
        nc.gpsimd.dma_start(wk_sb[:], wk.rearrange("p (t d) -> p t d", d=D))
        bkq_sb = persist.tile([P, 2], F32)
        nc.sync.dma_start(bkq_sb[:], bkq)
        bk_sb = bkq_sb[:, 0:1]
        bq_sb = bkq_sb[:, 1:2]
        bv_row = persist.tile([1, D], F32)
        nc.sync.dma_start(bv_row[:], bv)
        bv_bcast = persist.tile([P, D], F32)
        nc.gpsimd.partition_broadcast(bv_bcast[:], bv_row[:])
        x_r0 = xstage.tile([P, MT, SC], BF16, name="x_r")
        nc.sync.dma_start(x_r0[:, 0:MT // 2, :],
                          xT_r[:, 0:MT // 2, bass.ds(0, SC)])
        nc.sync.dma_start(x_r0[:, MT // 2:, :],
                          xT_r[:, MT // 2:, bass.ds(0, SC)])
        wq_sb = persist.tile([P, MT, D], BF16)
        nc.sync.dma_start(wq_sb[:], wq.rearrange("p (t d) -> p t d", d=D))
        wv_sb = persist.tile([P, MT, D], BF16)
        nc.sync.dma_start(wv_sb[:], wv.rearrange("p (t d) -> p t d", d=D))

        kT_sb = persist.tile([P, KH], BF16)        # K^T  [dk, s]
        qT_sb = persist.tile([P, S_all], BF16)     # Q^T  [dk, q]
        v_sb = persist.tile([P, ST, D], BF16)      # V    [s%128, s-tile, dv]

        apool = ctx.enter_context(tc.tile_pool(name="apool", bufs=5))
        dpool = ctx.enter_context(tc.tile_pool(name="dpool", bufs=2))
        otpool = ctx.enter_context(tc.tile_pool(name="otpool", bufs=2))
        spsum = ctx.enter_context(tc.tile_pool(name="spsum", bufs=2, space="PSUM"))
        opsum = ctx.enter_context(tc.tile_pool(name="opsum", bufs=2, space="PSUM"))

        a_t = {}      # qc -> A^T tile [P, ST, QC] bf16
        o_ps = {}     # qc -> N^T psum [P, QC]
        t1 = {}       # qc -> den partial [P, 4, QC] bf16
        dall = {}     # qc -> all-reduced partial denominator [P, QC] f32
        tail = {}     # LQ's PE-ones den psum [1, QC]

        def qsl(qc):
            return bass.ds(qc * QC, QC)

        def emit_scores(qc, pr, split_exp=False):
            """Scores for s-tiles (2pr, 2pr+1) x q-chunk qc + one wide exp."""
            if pr == 0:
                a_t[qc] = apool.tile([P, ST, QC], BF16, name="a_sb")
            ps_s = spsum.tile([P, 2, QC], F32, name="ps_s")
            for j in range(2):
                st = 2 * pr + j
                nc.tensor.matmul(ps_s[:, j, :], kT_sb[:, bass.ts(st, P)],
                                 qT_sb[:, qsl(qc)], start=True, stop=True)
            if split_exp:
                for j in range(2):
                    st = 2 * pr + j
                    nc.scalar.activation(a_t[qc][:, st:st + 1, :],
                                         ps_s[:, j:j + 1, :], Exp,
                                         scale=float(SCALE))
            else:
                nc.scalar.activation(a_t[qc][:, 2 * pr:2 * pr + 2, :], ps_s[:],
                                     Exp, scale=float(SCALE))
            # denominator tree triggers (tree q-chunks only)
            a = a_t[qc]
            if qc != LQ:
                if pr == 3:
                    t1[qc] = dpool.tile([P, 4, QC], BF16, name="t1")
                    nc.vector.tensor_add(t1[qc][:], a[:, 0:4, :], a[:, 4:8, :])
                elif pr == 7:
                    tb = dpool.tile([P, 4, QC], BF16, name="tb")
                    nc.vector.tensor_add(tb[:], a[:, 8:12, :], a[:, 12:16, :])
                    nc.vector.tensor_add(t1[qc][:], t1[qc][:], tb[:])
                    nc.vector.tensor_add(t1[qc][:, 0:2, :], t1[qc][:, 0:2, :],
                                         t1[qc][:, 2:4, :])
                    den128 = dpool.tile([P, QC], F32, name="den128")
                    nc.vector.tensor_add(den128[:], t1[qc][:, 0, :],
                                         t1[qc][:, 1, :])
                    dl = dpool.tile([P, QC], F32, name="dall")
                    nc.gpsimd.partition_all_reduce(dl[:], den128[:], P,
                                                   bass.bass_isa.ReduceOp.add)
                    dall[qc] = dl
                    nc.sync.dma_start(den[:, qsl(qc)], dl[:1, :])

        def emit_av(qc, pr):
            """attn@V accumulation for pair pr; the last q-chunk also feeds
            the PE ones-matmul denominator."""
            if qc not in o_ps:
                o_ps[qc] = opsum.tile([P, QC], F32, name="o_ps")
            for j in range(2):
                st = 2 * pr + j
                nc.tensor.matmul(o_ps[qc][:], v_sb[:, st, :],
                                 a_t[qc][:, st, :],
                                 start=(st == 0), stop=(st == ST - 1))
            if qc == LQ:
                if pr == 0:
                    tail[qc] = opsum.tile([1, QC], F32, name="o_ps")
                for j in range(2):
                    st = 2 * pr + j
                    nc.tensor.matmul(tail[qc][:], onesb[:], a_t[qc][:, st, :],
                                     start=(st == 0), stop=(st == ST - 1))

        def finish_qc(qc):
            """Drain N^T to SBUF and DMA it out; LQ also drains its PE-ones
            denominator (on ACT, parallel to the DVE drain)."""
            oT = otpool.tile([P, QC], BF16, name="oT")
            if qc == LQ:
                tail_sb = dpool.tile([1, QC], F32, name="tail_sb", bufs=1)
                nc.scalar.copy(tail_sb[:], tail[qc][:])
                nc.sync.dma_start(den[:, qsl(qc)], tail_sb[:])
            nc.vector.tensor_copy(oT[:], o_ps[qc][:])
            nc.sync.dma_start(out[:, qsl(qc)], oT[:])

        # scheduler state: two q-chunks own the O-psum banks at a time; the
        # rest chase scores only and catch up attn@V when promoted.
        sc_done = [0] * NQC
        av_done = [0] * NQC
        full = [0, 1]
        next_full = [2]

        def pump_avs(qc):
            if qc not in full:
                return
            while av_done[qc] < sc_done[qc] - (1 if sc_done[qc] < NPAIR else 0):
                emit_av(qc, av_done[qc])
                av_done[qc] += 1
            if av_done[qc] == NPAIR:
                finish_qc(qc)
                full.remove(qc)
                if next_full[0] < NQC:
                    nq = next_full[0]
                    next_full[0] += 1
                    full.append(nq)
                    pump_avs(nq)

        # PE warm-up: back-to-back transposes ramp the PE p-state while the
        # first x chunk lands.
        with tc.tile_pool(name="warm", bufs=1, space="PSUM") as wp:
            warm_ps = wp.tile([P, P], BF16, name="warm_ps")
            for _ in range(34):
                nc.tensor.transpose(warm_ps[:], identb[:], identb[:])

        with tc.tile_pool(name="pp", bufs=2, space="PSUM") as pp:
            for sc in range(NSC):
                ssl = bass.ds(sc * SC, SC)
                if sc == 0:
                    x_r = x_r0
                else:
                    x_r = xstage.tile([P, MT, SC], BF16, name="x_r")
                    nc.sync.dma_start(x_r[:], xT_r[:, :, ssl])

                if sc < NKC:
                    # K^T chunk (own key half = permuted row prefix)
                    ps = pp.tile([P, SC], F32, name="pp")
                    for mt in range(MT):
                        nc.tensor.matmul(ps[:], wk_sb[:, mt, :], x_r[:, mt, :],
                                         start=(mt == 0), stop=(mt == MT - 1))
                    nc.scalar.activation(kT_sb[:, ssl], ps[:], Ident,
                                         bias=bk_sb)

                # Q^T chunk (all 8 chunks)
                ps2 = pp.tile([P, SC], F32, name="pp")
                for mt in range(MT):
                    nc.tensor.matmul(ps2[:], wq_sb[:, mt, :], x_r[:, mt, :],
                                     start=(mt == 0), stop=(mt == MT - 1))
                nc.scalar.activation(qT_sb[:, ssl], ps2[:], Ident, bias=bq_sb)

                if sc < NKC:
                    # V chunk in natural [s, dv] layout: x-tile stationary
                    for t in range(SCT):
                        st = sc * SCT + t
                        psv = pp.tile([P, D], F32, name="pp")
                        for mt in range(MT):
                            nc.tensor.matmul(psv[:], x_r[:, mt, bass.ts(t, P)],
                                             wv_sb[:, mt, :],
                                             start=(mt == 0),
                                             stop=(mt == MT - 1))
                        nc.vector.tensor_add(v_sb[:, st, :], psv[:],
                                             bv_bcast[:])

                # chase: all q-chunks whose Q is projected catch up on all
                # available score pairs; slot owners also run attn@V
                avail = min(NKC, sc + 1) * SCT // 2
                for qc in range(NQC):
                    if qc <= sc:
                        while sc_done[qc] < avail:
                            pr = sc_done[qc]
                            emit_scores(qc, pr,
                                        split_exp=(qc == LQ and
                                                   pr == NPAIR - 1))
                            sc_done[qc] += 1
                            if qc in full:
                                pump_avs(qc)

        # post-stream: everything has its scores emitted; drain the rest
        for qc in list(full):
            pump_avs(qc)
        while full:
            qc = full[0]
            pump_avs(qc)
        ctx.close()

    return nc


def build(n_cores=N_CORES, **kw):
    nc = bacc.Bacc("TRN2", target_bir_lowering=False, debug=False,
                   num_devices=n_cores)
    build_attention(nc, **kw)
    nc.compile()
    return nc


def shard_inputs(input, Wq, bq, Wk, bk, Wv, bv):
    """Per-core in_maps. Core c: batch c//2, key-half c%2; the host permutes
    the batch so the core's key rows come first, transposes to xT [M, S],
    and converts to bf16. Weights are packed [P, MT*D]; biases packed small."""
    import ml_dtypes
    half = S // 2
    MT = M // 128

    def pack_w(W):
        return np.ascontiguousarray(
            np.asarray(W, dtype=np.float32).reshape(MT, 128, D)
            .transpose(1, 0, 2).reshape(128, MT * D)).astype(ml_dtypes.bfloat16)

    wq_b, wk_b, wv_b = pack_w(Wq), pack_w(Wk), pack_w(Wv)
    bkq_f = np.ascontiguousarray(np.stack(
        [np.asarray(bk, dtype=np.float32).ravel(),
         np.asarray(bq, dtype=np.float32).ravel()], axis=1))
    bv_f = np.asarray(bv, dtype=np.float32).reshape(1, D)
    in_maps = []
    for c in range(N_CORES):
        b, h = divmod(c, 2)
        xb = np.asarray(input[b])
        x_perm = np.concatenate(
            [xb[h * half:(h + 1) * half], xb[(1 - h) * half:(2 - h) * half]],
            axis=0)
        xT = np.ascontiguousarray(x_perm.T).astype(ml_dtypes.bfloat16)
        in_maps.append({
            "xT": xT,
            "wq": wq_b, "wk": wk_b, "wv": wv_b,
            "bkq": bkq_f, "bv": bv_f,
        })
    return in_maps


_NC_CACHE = {}


def kernel(input, Wq, bq, Wk, bk, Wv, bv):
    in_maps = shard_inputs(input, Wq, bq, Wk, bk, Wv, bv)
    if "nc" not in _NC_CACHE:
        _NC_CACHE["nc"] = build()
    nc = _NC_CACHE["nc"]
    res = run_bass_kernel_spmd(nc, in_maps, core_ids=list(range(N_CORES)))
    half = S // 2
    result = np.empty((B, S, D), dtype=np.float32)
    for b in range(B):
        # core 2b: keys half0, rows in natural order
        # core 2b+1: keys half1, rows permuted [half1; half0]
        n0 = np.asarray(res.results[2 * b]["out"]).astype(np.float32).T
        d0 = np.asarray(res.results[2 * b]["den"]).astype(np.float32).ravel()
        n1p = np.asarray(res.results[2 * b + 1]["out"]).astype(np.float32).T
        d1p = np.asarray(res.results[2 * b + 1]["den"]).astype(np.float32).ravel()
        n1 = np.concatenate([n1p[half:], n1p[:half]], axis=0)
        d1 = np.concatenate([d1p[half:], d1p[:half]])
        result[b] = (n0 + n1) / (d0 + d1)[:, None]
    return result


if __name__ == "__main__":
    rng = np.random.default_rng(0)
    inputs = {
        "input": rng.standard_normal((B, S, M), dtype=np.float32),
        "Wq": (rng.standard_normal((M, D), dtype=np.float32) / np.sqrt(M)).astype(np.float32),
        "bq": (rng.standard_normal(D, dtype=np.float32) * 0.02),
        "Wk": (rng.standard_normal((M, D), dtype=np.float32) / np.sqrt(M)).astype(np.float32),
        "bk": (rng.standard_normal(D, dtype=np.float32) * 0.02),
        "Wv": (rng.standard_normal((M, D), dtype=np.float32) / np.sqrt(M)).astype(np.float32),
        "bv": (rng.standard_normal(D, dtype=np.float32) * 0.02),
    }
    out = kernel(**inputs)
    print("kernel output:", out.shape, out.dtype)


# revision 50
# speedup vs baseline: 1.4549x; 1.0356x over previous
"""Self-contained Trainium2 Bass kernel for a single attention head.

Reference computation (per batch b):
    Q = x @ Wq + bq ; K = x @ Wk + bk ; V = x @ Wv + bv      (x: [S, M])
    out = softmax(Q K^T / sqrt(D)) @ V                        ([S, D])

Shapes: B=4, S=4096, M=1024, D=128, f32.

Sharding (key-split + host merge): 8 cores; core c handles batch b=c//2 and
KEY-half h=c%2. Each core projects K/V for its own 2048 key rows only, Q for
all 4096 queries, and computes the UNNORMALIZED partial attention
  N_h^T[dv, q] = sum_{s in half h} exp(q.k_s/sqrt(D)) v_s,   d_h[q] = sum_s exp(.)
over its key half. The host merges: O = (N_0 + N_1) / (d_0 + d_1) — softmax
over the key axis is an exact sum-decomposition, so the merge is exact. This
halves the redundant K/V projection work and removes every on-device
normalization/transpose step (the host divides and transposes). The host
permutes each core's rows so its key half comes first (key order inside a
softmax is irrelevant; the query order is un-permuted on the host).

Device pipeline (fully fused stream over 8 x-chunks of 512 rows):
  - chunks 0-3 project K^T/V (own keys) + Q^T; chunks 4-7 project Q^T only.
    x is bf16 (host-converted; halves DMA). V is built in natural [s, dv]
    layout directly via x-tile-stationary matmuls (no transposes).
  - 8 q-chunks chase the stream: scores S^T[s,q] (bf16 matmuls into PSUM),
    one wide exp per s-tile pair [128,1024] -> A^T bf16, attn@V accumulates
    N^T in PSUM. Two q-chunks hold the two O-psum banks at a time; the
    others defer attn@V until a slot frees (their A^T stays in SBUF).
  - denominator: per q-chunk DVE bf16 tree -> f32 -> GPSIMD
    partition_all_reduce -> DMA; the last q-chunk accumulates its den with
    PE ones-matmuls instead so the post-last-exp tail is tiny.
  - N^T q-slabs DMA out straight from the PSUM drain; no finalize pass.
  - PSUM (8 banks): scores 2x2 + N^T accumulators 2 + projections 2.
"""

from contextlib import ExitStack

import numpy as np

import concourse.bass as bass
import concourse.tile as tile
from concourse import bacc, mybir
from concourse.bass_utils import run_bass_kernel_spmd
from concourse.masks import make_identity

F32 = mybir.dt.float32
BF16 = mybir.dt.bfloat16

B, S, M, D = 4, 4096, 1024, 128
N_CORES = 8
P = 128
SCALE = 1.0 / np.sqrt(np.float32(D))


def build_attention(nc, S_all=S, M_dim=M, SC=512, QC=512):
    KH = S_all // 2               # keys per core (2048)
    MT = M_dim // P               # m-tiles (8)
    ST = KH // P                  # key s-tiles (16)
    NSC = S_all // SC             # x-chunks (8)
    NKC = KH // SC                # key chunks (4)
    SCT = SC // P                 # s-tiles per chunk (4)
    NQC = S_all // QC             # q-chunks (8)
    NPAIR = ST // 2               # score pairs per q-chunk (8)
    LQ = NQC - 1                  # last q-chunk (PE-ones denominator)

    xT = nc.dram_tensor("xT", [M_dim, S_all], BF16, kind="ExternalInput").ap()
    wq = nc.dram_tensor("wq", [P, MT * D], BF16, kind="ExternalInput").ap()
    wk = nc.dram_tensor("wk", [P, MT * D], BF16, kind="ExternalInput").ap()
    wv = nc.dram_tensor("wv", [P, MT * D], BF16, kind="ExternalInput").ap()
    bkq = nc.dram_tensor("bkq", [P, 2], F32, kind="ExternalInput").ap()
    bv = nc.dram_tensor("bv", [1, D], F32, kind="ExternalInput").ap()
    out = nc.dram_tensor("out", [D, S_all], BF16, kind="ExternalOutput").ap()
    den = nc.dram_tensor("den", [1, S_all], F32, kind="ExternalOutput").ap()

    xT_r = xT.rearrange("(t p) s -> p t s", p=P)

    Ident = mybir.ActivationFunctionType.Identity
    Exp = mybir.ActivationFunctionType.Exp

    with tile.TileContext(nc) as tc:
        ctx = ExitStack()
        persist = ctx.enter_context(tc.tile_pool(name="persist", bufs=1))

        ident = persist.tile([P, P], F32)
        make_identity(nc, ident[:])
        identb = persist.tile([P, P], BF16)
        nc.vector.tensor_copy(identb[:], ident[:])
        ones_f = persist.tile([P, 1], F32)
        nc.vector.memset(ones_f[:], 1.0)
        onesb = persist.tile([P, 1], BF16)
        nc.vector.tensor_copy(onesb[:], ones_f[:])

        # startup DMA order: wk (gpsimd queue, instant issue), then biases +
        # x chunk 0 halves + wq/wv on the SP queue in priority order
        xstage = ctx.enter_context(tc.tile_pool(name="xstage", bufs=2))
        wk_sb = persist.tile([P, MT, D], BF16)
        nc.gpsimd.dma_start(wk_sb[:], wk.rearrange("p (t d) -> p t d", d=D))
        bkq_sb = persist.tile([P, 2], F32)
        nc.sync.dma_start(bkq_sb[:], bkq)
        bk_sb = bkq_sb[:, 0:1]
        bq_sb = bkq_sb[:, 1:2]
        bv_row = persist.tile([1, D], F32)
        nc.sync.dma_start(bv_row[:], bv)
        bv_bcast = persist.tile([P, D], F32)
        nc.gpsimd.partition_broadcast(bv_bcast[:], bv_row[:])
        x_r0 = xstage.tile([P, MT, SC], BF16, name="x_r")
        nc.sync.dma_start(x_r0[:, 0:MT // 2, :],
                          xT_r[:, 0:MT // 2, bass.ds(0, SC)])
        nc.sync.dma_start(x_r0[:, MT // 2:, :],
                          xT_r[:, MT // 2:, bass.ds(0, SC)])
        wq_sb = persist.tile([P, MT, D], BF16)
        nc.sync.dma_start(wq_sb[:], wq.rearrange("p (t d) -> p t d", d=D))
        wv_sb = persist.tile([P, MT, D], BF16)
        nc.sync.dma_start(wv_sb[:], wv.rearrange("p (t d) -> p t d", d=D))

        kT_sb = persist.tile([P, KH], BF16)        # K^T  [dk, s]
        qT_sb = persist.tile([P, S_all], BF16)     # Q^T  [dk, q]
        v_sb = persist.tile([P, ST, D], BF16)      # V    [s%128, s-tile, dv]

        apool = ctx.enter_context(tc.tile_pool(name="apool", bufs=5))
        dpool = ctx.enter_context(tc.tile_pool(name="dpool", bufs=2))
        otpool = ctx.enter_context(tc.tile_pool(name="otpool", bufs=2))
        spsum = ctx.enter_context(tc.tile_pool(name="spsum", bufs=2, space="PSUM"))
        opsum = ctx.enter_context(tc.tile_pool(name="opsum", bufs=2, space="PSUM"))

        a_t = {}      # qc -> A^T tile [P, ST, QC] bf16
        o_ps = {}     # qc -> N^T psum [P, QC]
        t1 = {}       # qc -> den partial [P, 4, QC] bf16
        dall = {}     # qc -> all-reduced partial denominator [P, QC] f32
        tail = {}     # LQ's PE-ones den psum [1, QC]

        def qsl(qc):
            return bass.ds(qc * QC, QC)

        def emit_scores(qc, pr, split_exp=False):
            """Scores for s-tiles (2pr, 2pr+1) x q-chunk qc + one wide exp."""
            if pr == 0:
                a_t[qc] = apool.tile([P, ST, QC], BF16, name="a_sb")
            ps_s = spsum.tile([P, 2, QC], F32, name="ps_s")
            for j in range(2):
                st = 2 * pr + j
                nc.tensor.matmul(ps_s[:, j, :], kT_sb[:, bass.ts(st, P)],
                                 qT_sb[:, qsl(qc)], start=True, stop=True)
            if split_exp:
                for j in range(2):
                    st = 2 * pr + j
                    nc.scalar.activation(a_t[qc][:, st:st + 1, :],
                                         ps_s[:, j:j + 1, :], Exp,
                                         scale=float(SCALE))
            else:
                nc.scalar.activation(a_t[qc][:, 2 * pr:2 * pr + 2, :], ps_s[:],
                                     Exp, scale=float(SCALE))
            # denominator tree triggers (tree q-chunks only)
            a = a_t[qc]
            if qc != LQ:
                if pr == 3:
                    t1[qc] = dpool.tile([P, 4, QC], BF16, name="t1")
                    nc.vector.tensor_add(t1[qc][:], a[:, 0:4, :], a[:, 4:8, :])
                elif pr == 7:
                    tb = dpool.tile([P, 4, QC], BF16, name="tb")
                    nc.vector.tensor_add(tb[:], a[:, 8:12, :], a[:, 12:16, :])
                    nc.vector.tensor_add(t1[qc][:], t1[qc][:], tb[:])
                    nc.vector.tensor_add(t1[qc][:, 0:2, :], t1[qc][:, 0:2, :],
                                         t1[qc][:, 2:4, :])
                    den128 = dpool.tile([P, QC], F32, name="den128")
                    nc.vector.tensor_add(den128[:], t1[qc][:, 0, :],
                                         t1[qc][:, 1, :])
                    dl = dpool.tile([P, QC], F32, name="dall")
                    nc.gpsimd.partition_all_reduce(dl[:], den128[:], P,
                                                   bass.bass_isa.ReduceOp.add)
                    dall[qc] = dl
                    nc.sync.dma_start(den[:, qsl(qc)], dl[:1, :])

        def emit_av(qc, pr):
            """attn@V accumulation for pair pr; the last q-chunk also feeds
            the PE ones-matmul denominator."""
            if qc not in o_ps:
                o_ps[qc] = opsum.tile([P, QC], F32, name="o_ps")
            for j in range(2):
                st = 2 * pr + j
                nc.tensor.matmul(o_ps[qc][:], v_sb[:, st, :],
                                 a_t[qc][:, st, :],
                                 start=(st == 0), stop=(st == ST - 1))
            if qc == LQ:
                if pr == 0:
                    tail[qc] = opsum.tile([1, QC], F32, name="o_ps")
                for j in range(2):
                    st = 2 * pr + j
                    nc.tensor.matmul(tail[qc][:], onesb[:], a_t[qc][:, st, :],
                                     start=(st == 0), stop=(st == ST - 1))

        def finish_qc(qc):
            """Drain N^T to SBUF and DMA it out; LQ also drains its PE-ones
            denominator (on ACT, parallel to the DVE drain). The last drain
            and output DMA are split in halves so the first transfer's launch
            latency overlaps the second half's drain."""
            oT = otpool.tile([P, QC], BF16, name="oT")
            if qc == LQ:
                tail_sb = dpool.tile([1, QC], F32, name="tail_sb", bufs=1)
                nc.scalar.copy(tail_sb[:], tail[qc][:])
                nc.gpsimd.dma_start(den[:, qsl(qc)], tail_sb[:])
                h = QC // 2
                nc.vector.tensor_copy(oT[:, 0:h], o_ps[qc][:, 0:h])
                nc.sync.dma_start(out[:, bass.ds(qc * QC, h)], oT[:, 0:h])
                nc.vector.tensor_copy(oT[:, h:], o_ps[qc][:, h:])
                # second half on the ACT queue so its launch overlaps the first
                nc.scalar.dma_start(out[:, bass.ds(qc * QC + h, h)], oT[:, h:])
            else:
                nc.vector.tensor_copy(oT[:], o_ps[qc][:])
                nc.sync.dma_start(out[:, qsl(qc)], oT[:])

        # scheduler state: two q-chunks own the O-psum banks at a time; the
        # rest chase scores only and catch up attn@V when promoted.
        sc_done = [0] * NQC
        av_done = [0] * NQC
        full = [0, 1]
        next_full = [2]

        def pump_avs(qc):
            if qc not in full:
                return
            while av_done[qc] < sc_done[qc] - (1 if sc_done[qc] < NPAIR else 0):
                emit_av(qc, av_done[qc])
                av_done[qc] += 1
            if av_done[qc] == NPAIR:
                finish_qc(qc)
                full.remove(qc)
                if next_full[0] < NQC:
                    nq = next_full[0]
                    next_full[0] += 1
                    full.append(nq)
                    pump_avs(nq)

        # PE warm-up: back-to-back transposes ramp the PE p-state while the
        # first x chunk lands.
        with tc.tile_pool(name="warm", bufs=1, space="PSUM") as wp:
            warm_ps = wp.tile([P, P], BF16, name="warm_ps")
            for _ in range(34):
                nc.tensor.transpose(warm_ps[:], identb[:], identb[:])

        with tc.tile_pool(name="pp", bufs=2, space="PSUM") as pp:
            for sc in range(NSC):
                ssl = bass.ds(sc * SC, SC)
                if sc == 0:
                    x_r = x_r0
                else:
                    x_r = xstage.tile([P, MT, SC], BF16, name="x_r")
                    nc.sync.dma_start(x_r[:], xT_r[:, :, ssl])

                if sc < NKC:
                    # K^T chunk (own key half = permuted row prefix)
                    ps = pp.tile([P, SC], F32, name="pp")
                    for mt in range(MT):
                        nc.tensor.matmul(ps[:], wk_sb[:, mt, :], x_r[:, mt, :],
                                         start=(mt == 0), stop=(mt == MT - 1))
                    nc.vector.tensor_scalar_add(kT_sb[:, ssl], ps[:], bk_sb)

                # Q^T chunk (all 8 chunks); late-chunk drains go on the DVE so
                # they don't stretch the exp cadence in the ACT-bound phase
                ps2 = pp.tile([P, SC], F32, name="pp")
                for mt in range(MT):
                    nc.tensor.matmul(ps2[:], wq_sb[:, mt, :], x_r[:, mt, :],
                                     start=(mt == 0), stop=(mt == MT - 1))
                nc.vector.tensor_scalar_add(qT_sb[:, ssl], ps2[:], bq_sb)

                if sc < NKC:
                    # V chunk in natural [s, dv] layout: x-tile stationary
                    for t in range(SCT):
                        st = sc * SCT + t
                        psv = pp.tile([P, D], F32, name="pp")
                        for mt in range(MT):
                            nc.tensor.matmul(psv[:], x_r[:, mt, bass.ts(t, P)],
                                             wv_sb[:, mt, :],
                                             start=(mt == 0),
                                             stop=(mt == MT - 1))
                        nc.vector.tensor_add(v_sb[:, st, :], psv[:],
                                             bv_bcast[:])

                # chase: all q-chunks whose Q is projected catch up on all
                # available score pairs; slot owners also run attn@V
                avail = min(NKC, sc + 1) * SCT // 2
                for qc in range(NQC):
                    if qc <= sc:
                        while sc_done[qc] < avail:
                            pr = sc_done[qc]
                            emit_scores(qc, pr,
                                        split_exp=(qc == LQ and
                                                   pr == NPAIR - 1))
                            sc_done[qc] += 1
                            if qc in full:
                                pump_avs(qc)

        # post-stream: everything has its scores emitted; drain the rest
        for qc in list(full):
            pump_avs(qc)
        while full:
            qc = full[0]
            pump_avs(qc)
        ctx.close()

    return nc


def build(n_cores=N_CORES, **kw):
    nc = bacc.Bacc("TRN2", target_bir_lowering=False, debug=False,
                   num_devices=n_cores)
    build_attention(nc, **kw)
    nc.compile()
    return nc


def shard_inputs(input, Wq, bq, Wk, bk, Wv, bv):
    """Per-core in_maps. Core c: batch c//2, key-half c%2; the host permutes
    the batch so the core's key rows come first, transposes to xT [M, S],
    and converts to bf16. Weights are packed [P, MT*D]; biases packed small."""
    import ml_dtypes
    half = S // 2
    MT = M // 128

    def pack_w(W):
        return np.ascontiguousarray(
            np.asarray(W, dtype=np.float32).reshape(MT, 128, D)
            .transpose(1, 0, 2).reshape(128, MT * D)).astype(ml_dtypes.bfloat16)

    wq_b, wk_b, wv_b = pack_w(Wq), pack_w(Wk), pack_w(Wv)
    bkq_f = np.ascontiguousarray(np.stack(
        [np.asarray(bk, dtype=np.float32).ravel(),
         np.asarray(bq, dtype=np.float32).ravel()], axis=1))
    bv_f = np.asarray(bv, dtype=np.float32).reshape(1, D)
    in_maps = []
    for c in range(N_CORES):
        b, h = divmod(c, 2)
        xb = np.asarray(input[b])
        x_perm = np.concatenate(
            [xb[h * half:(h + 1) * half], xb[(1 - h) * half:(2 - h) * half]],
            axis=0)
        xT = np.ascontiguousarray(x_perm.T).astype(ml_dtypes.bfloat16)
        in_maps.append({
            "xT": xT,
            "wq": wq_b, "wk": wk_b, "wv": wv_b,
            "bkq": bkq_f, "bv": bv_f,
        })
    return in_maps


_NC_CACHE = {}


def kernel(input, Wq, bq, Wk, bk, Wv, bv):
    in_maps = shard_inputs(input, Wq, bq, Wk, bk, Wv, bv)
    if "nc" not in _NC_CACHE:
        _NC_CACHE["nc"] = build()
    nc = _NC_CACHE["nc"]
    res = run_bass_kernel_spmd(nc, in_maps, core_ids=list(range(N_CORES)))
    half = S // 2
    result = np.empty((B, S, D), dtype=np.float32)
    for b in range(B):
        # core 2b: keys half0, rows in natural order
        # core 2b+1: keys half1, rows permuted [half1; half0]
        n0 = np.asarray(res.results[2 * b]["out"]).astype(np.float32).T
        d0 = np.asarray(res.results[2 * b]["den"]).astype(np.float32).ravel()
        n1p = np.asarray(res.results[2 * b + 1]["out"]).astype(np.float32).T
        d1p = np.asarray(res.results[2 * b + 1]["den"]).astype(np.float32).ravel()
        n1 = np.concatenate([n1p[half:], n1p[:half]], axis=0)
        d1 = np.concatenate([d1p[half:], d1p[:half]])
        result[b] = (n0 + n1) / (d0 + d1)[:, None]
    return result


if __name__ == "__main__":
    rng = np.random.default_rng(0)
    inputs = {
        "input": rng.standard_normal((B, S, M), dtype=np.float32),
        "Wq": (rng.standard_normal((M, D), dtype=np.float32) / np.sqrt(M)).astype(np.float32),
        "bq": (rng.standard_normal(D, dtype=np.float32) * 0.02),
        "Wk": (rng.standard_normal((M, D), dtype=np.float32) / np.sqrt(M)).astype(np.float32),
        "bk": (rng.standard_normal(D, dtype=np.float32) * 0.02),
        "Wv": (rng.standard_normal((M, D), dtype=np.float32) / np.sqrt(M)).astype(np.float32),
        "bv": (rng.standard_normal(D, dtype=np.float32) * 0.02),
    }
    out = kernel(**inputs)
    print("kernel output:", out.shape, out.dtype)


# revision 53
# speedup vs baseline: 1.4664x; 1.0079x over previous
"""Self-contained Trainium2 Bass kernel for a single attention head.

Reference computation (per batch b):
    Q = x @ Wq + bq ; K = x @ Wk + bk ; V = x @ Wv + bv      (x: [S, M])
    out = softmax(Q K^T / sqrt(D)) @ V                        ([S, D])

Shapes: B=4, S=4096, M=1024, D=128, f32.

Sharding (key-split + host merge): 8 cores; core c handles batch b=c//2 and
KEY-half h=c%2. Each core projects K/V for its own 2048 key rows only, Q for
all 4096 queries, and computes the UNNORMALIZED partial attention
  N_h^T[dv, q] = sum_{s in half h} exp(q.k_s/sqrt(D)) v_s,   d_h[q] = sum_s exp(.)
over its key half. The host merges: O = (N_0 + N_1) / (d_0 + d_1) — softmax
over the key axis is an exact sum-decomposition, so the merge is exact. This
halves the redundant K/V projection work and removes every on-device
normalization/transpose step (the host divides and transposes). The host
permutes each core's rows so its key half comes first (key order inside a
softmax is irrelevant; the query order is un-permuted on the host).

Device pipeline (fully fused stream over 8 x-chunks of 512 rows):
  - chunks 0-3 project K^T/V (own keys) + Q^T; chunks 4-7 project Q^T only.
    x is bf16 (host-converted; halves DMA). V is built in natural [s, dv]
    layout directly via x-tile-stationary matmuls (no transposes).
  - 8 q-chunks chase the stream: scores S^T[s,q] (bf16 matmuls into PSUM),
    one wide exp per s-tile pair [128,1024] -> A^T bf16, attn@V accumulates
    N^T in PSUM. Two q-chunks hold the two O-psum banks at a time; the
    others defer attn@V until a slot frees (their A^T stays in SBUF).
  - denominator: per q-chunk DVE bf16 tree -> f32 -> GPSIMD
    partition_all_reduce -> DMA; the last q-chunk accumulates its den with
    PE ones-matmuls instead so the post-last-exp tail is tiny.
  - N^T q-slabs DMA out straight from the PSUM drain; no finalize pass.
  - PSUM (8 banks): scores 2x2 + N^T accumulators 2 + projections 2.
"""

from contextlib import ExitStack

import numpy as np

import concourse.bass as bass
import concourse.tile as tile
from concourse import bacc, mybir
from concourse.bass_utils import run_bass_kernel_spmd
from concourse.masks import make_identity

F32 = mybir.dt.float32
BF16 = mybir.dt.bfloat16

B, S, M, D = 4, 4096, 1024, 128
N_CORES = 8
P = 128
SCALE = 1.0 / np.sqrt(np.float32(D))


def build_attention(nc, S_all=S, M_dim=M, SC=512, QC=512):
    KH = S_all // 2               # keys per core (2048)
    MT = M_dim // P               # m-tiles (8)
    ST = KH // P                  # key s-tiles (16)
    NSC = S_all // SC             # x-chunks (8)
    NKC = KH // SC                # key chunks (4)
    SCT = SC // P                 # s-tiles per chunk (4)
    NQC = S_all // QC             # q-chunks (8)
    NPAIR = ST // 2               # score pairs per q-chunk (8)
    LQ = NQC - 1                  # last q-chunk (PE-ones denominator)

    xT = nc.dram_tensor("xT", [M_dim, S_all], BF16, kind="ExternalInput").ap()
    wq = nc.dram_tensor("wq", [P, MT * D], BF16, kind="ExternalInput").ap()
    wk = nc.dram_tensor("wk", [P, MT * D], BF16, kind="ExternalInput").ap()
    wv = nc.dram_tensor("wv", [P, MT * D], BF16, kind="ExternalInput").ap()
    bkq = nc.dram_tensor("bkq", [P, 2], F32, kind="ExternalInput").ap()
    bv = nc.dram_tensor("bv", [1, D], F32, kind="ExternalInput").ap()
    out = nc.dram_tensor("out", [D, S_all], BF16, kind="ExternalOutput").ap()
    den = nc.dram_tensor("den", [1, S_all], F32, kind="ExternalOutput").ap()

    xT_r = xT.rearrange("(t p) s -> p t s", p=P)

    Ident = mybir.ActivationFunctionType.Identity
    Exp = mybir.ActivationFunctionType.Exp

    with tile.TileContext(nc) as tc:
        ctx = ExitStack()
        persist = ctx.enter_context(tc.tile_pool(name="persist", bufs=1))

        ident = persist.tile([P, P], F32)
        make_identity(nc, ident[:])
        identb = persist.tile([P, P], BF16)
        nc.vector.tensor_copy(identb[:], ident[:])
        ones_f = persist.tile([P, 1], F32)
        nc.vector.memset(ones_f[:], 1.0)
        onesb = persist.tile([P, 1], BF16)
        nc.vector.tensor_copy(onesb[:], ones_f[:])

        # startup DMA order: wk (gpsimd queue, instant issue), then biases +
        # x chunk 0 halves + wq/wv on the SP queue in priority order
        xstage = ctx.enter_context(tc.tile_pool(name="xstage", bufs=2))
        wk_sb = persist.tile([P, MT, D], BF16)
        nc.gpsimd.dma_start(wk_sb[:], wk.rearrange("p (t d) -> p t d", d=D))
        bkq_sb = persist.tile([P, 2], F32)
        nc.sync.dma_start(bkq_sb[:], bkq)
        bk_sb = bkq_sb[:, 0:1]
        bq_sb = bkq_sb[:, 1:2]
        bv_row = persist.tile([1, D], F32)
        nc.sync.dma_start(bv_row[:], bv)
        bv_bcast = persist.tile([P, D], F32)
        nc.gpsimd.partition_broadcast(bv_bcast[:], bv_row[:])
        x_r0 = xstage.tile([P, MT, SC], BF16, name="x_r")
        nc.sync.dma_start(x_r0[:, 0:MT // 2, :],
                          xT_r[:, 0:MT // 2, bass.ds(0, SC)])
        nc.sync.dma_start(x_r0[:, MT // 2:, :],
                          xT_r[:, MT // 2:, bass.ds(0, SC)])
        wq_sb = persist.tile([P, MT, D], BF16)
        nc.sync.dma_start(wq_sb[:], wq.rearrange("p (t d) -> p t d", d=D))
        wv_sb = persist.tile([P, MT, D], BF16)
        nc.sync.dma_start(wv_sb[:], wv.rearrange("p (t d) -> p t d", d=D))

        kT_sb = persist.tile([P, KH], BF16)        # K^T  [dk, s]
        qT_sb = persist.tile([P, S_all], BF16)     # Q^T  [dk, q]
        v_sb = persist.tile([P, ST, D], BF16)      # V    [s%128, s-tile, dv]

        apool = ctx.enter_context(tc.tile_pool(name="apool", bufs=5))
        dpool = ctx.enter_context(tc.tile_pool(name="dpool", bufs=2))
        otpool = ctx.enter_context(tc.tile_pool(name="otpool", bufs=2))
        spsum = ctx.enter_context(tc.tile_pool(name="spsum", bufs=2, space="PSUM"))
        opsum = ctx.enter_context(tc.tile_pool(name="opsum", bufs=2, space="PSUM"))

        a_t = {}      # qc -> A^T tile [P, ST, QC] bf16
        o_ps = {}     # qc -> N^T psum [P, QC]
        t1 = {}       # qc -> den partial [P, 4, QC] bf16
        dall = {}     # qc -> all-reduced partial denominator [P, QC] f32
        tail = {}     # LQ's PE-ones den psum [1, QC]

        def qsl(qc):
            return bass.ds(qc * QC, QC)

        def emit_scores(qc, pr, split_exp=False):
            """Scores for s-tiles (2pr, 2pr+1) x q-chunk qc + one wide exp."""
            if pr == 0:
                a_t[qc] = apool.tile([P, ST, QC], BF16, name="a_sb")
            ps_s = spsum.tile([P, 2, QC], F32, name="ps_s")
            for j in range(2):
                st = 2 * pr + j
                nc.tensor.matmul(ps_s[:, j, :], kT_sb[:, bass.ts(st, P)],
                                 qT_sb[:, qsl(qc)], start=True, stop=True)
            if split_exp:
                for j in range(2):
                    st = 2 * pr + j
                    nc.scalar.activation(a_t[qc][:, st:st + 1, :],
                                         ps_s[:, j:j + 1, :], Exp,
                                         scale=float(SCALE))
            else:
                nc.scalar.activation(a_t[qc][:, 2 * pr:2 * pr + 2, :], ps_s[:],
                                     Exp, scale=float(SCALE))
            # denominator tree triggers. LQ folds tiles 0..11 early (its last
            # 4 tiles ride the PE ones-matmuls so the end tail stays short).
            a = a_t[qc]
            if pr == 3:
                t1[qc] = dpool.tile([P, 4, QC], BF16, name="t1")
                nc.vector.tensor_add(t1[qc][:], a[:, 0:4, :], a[:, 4:8, :])
            elif qc != LQ and pr == 7:
                tb = dpool.tile([P, 4, QC], BF16, name="tb")
                nc.vector.tensor_add(tb[:], a[:, 8:12, :], a[:, 12:16, :])
                nc.vector.tensor_add(t1[qc][:], t1[qc][:], tb[:])
                _den_fold(qc)
                nc.sync.dma_start(den[:, qsl(qc)], dall[qc][:1, :])
            elif qc == LQ and pr == 5:
                nc.vector.tensor_add(t1[qc][:], t1[qc][:], a[:, 8:12, :])
                _den_fold(qc)

        def _den_fold(qc):
            nc.vector.tensor_add(t1[qc][:, 0:2, :], t1[qc][:, 0:2, :],
                                 t1[qc][:, 2:4, :])
            den128 = dpool.tile([P, QC], F32, name="den128")
            nc.vector.tensor_add(den128[:], t1[qc][:, 0, :], t1[qc][:, 1, :])
            dl = dpool.tile([P, QC], F32, name="dall")
            nc.gpsimd.partition_all_reduce(dl[:], den128[:], P,
                                           bass.bass_isa.ReduceOp.add)
            dall[qc] = dl

        def emit_av(qc, pr):
            """attn@V accumulation for pair pr; the last q-chunk's final four
            s-tiles also feed the PE ones-matmul denominator tail."""
            if qc not in o_ps:
                o_ps[qc] = opsum.tile([P, QC], F32, name="o_ps")
            for j in range(2):
                st = 2 * pr + j
                nc.tensor.matmul(o_ps[qc][:], v_sb[:, st, :],
                                 a_t[qc][:, st, :],
                                 start=(st == 0), stop=(st == ST - 1))
            if qc == LQ and pr >= NPAIR - 2:
                if pr == NPAIR - 2:
                    tail[qc] = opsum.tile([1, QC], F32, name="o_ps")
                for j in range(2):
                    st = 2 * pr + j
                    nc.tensor.matmul(tail[qc][:], onesb[:], a_t[qc][:, st, :],
                                     start=(st == ST - 4), stop=(st == ST - 1))

        def finish_qc(qc):
            """Drain N^T to SBUF and DMA it out; LQ also drains its PE-ones
            denominator (on ACT, parallel to the DVE drain). The last drain
            and output DMA are split in halves so the first transfer's launch
            latency overlaps the second half's drain."""
            oT = otpool.tile([P, QC], BF16, name="oT")
            if qc == LQ:
                tail_sb = dpool.tile([1, QC], F32, name="tail_sb", bufs=1)
                nc.scalar.copy(tail_sb[:], tail[qc][:])
                dcomb = dpool.tile([1, QC], F32, name="dcomb", bufs=1)
                nc.vector.tensor_add(dcomb[:], dall[qc][:1, :], tail_sb[:])
                nc.gpsimd.dma_start(den[:, qsl(qc)], dcomb[:])
                h = QC // 2
                nc.vector.tensor_copy(oT[:, 0:h], o_ps[qc][:, 0:h])
                nc.sync.dma_start(out[:, bass.ds(qc * QC, h)], oT[:, 0:h])
                nc.vector.tensor_copy(oT[:, h:], o_ps[qc][:, h:])
                # second half on the ACT queue so its launch overlaps the first
                nc.scalar.dma_start(out[:, bass.ds(qc * QC + h, h)], oT[:, h:])
            else:
                nc.vector.tensor_copy(oT[:], o_ps[qc][:])
                nc.sync.dma_start(out[:, qsl(qc)], oT[:])

        # scheduler state: two q-chunks own the O-psum banks at a time; the
        # rest chase scores only and catch up attn@V when promoted.
        sc_done = [0] * NQC
        av_done = [0] * NQC
        full = [0, 1]
        next_full = [2]

        def pump_avs(qc):
            if qc not in full:
                return
            while av_done[qc] < sc_done[qc] - (1 if sc_done[qc] < NPAIR else 0):
                emit_av(qc, av_done[qc])
                av_done[qc] += 1
            if av_done[qc] == NPAIR:
                finish_qc(qc)
                full.remove(qc)
                if next_full[0] < NQC:
                    nq = next_full[0]
                    next_full[0] += 1
                    full.append(nq)
                    pump_avs(nq)

        # PE warm-up: back-to-back transposes ramp the PE p-state while the
        # first x chunk lands.
        with tc.tile_pool(name="warm", bufs=1, space="PSUM") as wp:
            warm_ps = wp.tile([P, P], BF16, name="warm_ps")
            for _ in range(34):
                nc.tensor.transpose(warm_ps[:], identb[:], identb[:])

        with tc.tile_pool(name="pp", bufs=2, space="PSUM") as pp:
            for sc in range(NSC):
                ssl = bass.ds(sc * SC, SC)
                if sc == 0:
                    x_r = x_r0
                else:
                    x_r = xstage.tile([P, MT, SC], BF16, name="x_r")
                    nc.sync.dma_start(x_r[:], xT_r[:, :, ssl])

                if sc < NKC:
                    # K^T chunk (own key half = permuted row prefix)
                    ps = pp.tile([P, SC], F32, name="pp")
                    for mt in range(MT):
                        nc.tensor.matmul(ps[:], wk_sb[:, mt, :], x_r[:, mt, :],
                                         start=(mt == 0), stop=(mt == MT - 1))
                    nc.vector.tensor_scalar_add(kT_sb[:, ssl], ps[:], bk_sb)

                # Q^T chunk (all 8 chunks); late-chunk drains go on the DVE so
                # they don't stretch the exp cadence in the ACT-bound phase
                ps2 = pp.tile([P, SC], F32, name="pp")
                for mt in range(MT):
                    nc.tensor.matmul(ps2[:], wq_sb[:, mt, :], x_r[:, mt, :],
                                     start=(mt == 0), stop=(mt == MT - 1))
                nc.vector.tensor_scalar_add(qT_sb[:, ssl], ps2[:], bq_sb)

                if sc < NKC:
                    # V chunk in natural [s, dv] layout: x-tile stationary
                    for t in range(SCT):
                        st = sc * SCT + t
                        psv = pp.tile([P, D], F32, name="pp")
                        for mt in range(MT):
                            nc.tensor.matmul(psv[:], x_r[:, mt, bass.ts(t, P)],
                                             wv_sb[:, mt, :],
                                             start=(mt == 0),
                                             stop=(mt == MT - 1))
                        nc.vector.tensor_add(v_sb[:, st, :], psv[:],
                                             bv_bcast[:])

                # chase: all q-chunks whose Q is projected catch up on all
                # available score pairs; slot owners also run attn@V
                avail = min(NKC, sc + 1) * SCT // 2
                for qc in range(NQC):
                    if qc <= sc:
                        while sc_done[qc] < avail:
                            pr = sc_done[qc]
                            emit_scores(qc, pr,
                                        split_exp=(qc == LQ and
                                                   pr == NPAIR - 1))
                            sc_done[qc] += 1
                            if qc in full:
                                pump_avs(qc)

        # post-stream: everything has its scores emitted; drain the rest
        for qc in list(full):
            pump_avs(qc)
        while full:
            qc = full[0]
            pump_avs(qc)
        ctx.close()

    return nc


def build(n_cores=N_CORES, **kw):
    nc = bacc.Bacc("TRN2", target_bir_lowering=False, debug=False,
                   num_devices=n_cores)
    build_attention(nc, **kw)
    nc.compile()
    return nc


def shard_inputs(input, Wq, bq, Wk, bk, Wv, bv):
    """Per-core in_maps. Core c: batch c//2, key-half c%2; the host permutes
    the batch so the core's key rows come first, transposes to xT [M, S],
    and converts to bf16. Weights are packed [P, MT*D]; biases packed small."""
    import ml_dtypes
    half = S // 2
    MT = M // 128

    def pack_w(W):
        return np.ascontiguousarray(
            np.asarray(W, dtype=np.float32).reshape(MT, 128, D)
            .transpose(1, 0, 2).reshape(128, MT * D)).astype(ml_dtypes.bfloat16)

    wq_b, wk_b, wv_b = pack_w(Wq), pack_w(Wk), pack_w(Wv)
    bkq_f = np.ascontiguousarray(np.stack(
        [np.asarray(bk, dtype=np.float32).ravel(),
         np.asarray(bq, dtype=np.float32).ravel()], axis=1))
    bv_f = np.asarray(bv, dtype=np.float32).reshape(1, D)
    in_maps = []
    for c in range(N_CORES):
        b, h = divmod(c, 2)
        xb = np.asarray(input[b])
        x_perm = np.concatenate(
            [xb[h * half:(h + 1) * half], xb[(1 - h) * half:(2 - h) * half]],
            axis=0)
        xT = np.ascontiguousarray(x_perm.T).astype(ml_dtypes.bfloat16)
        in_maps.append({
            "xT": xT,
            "wq": wq_b, "wk": wk_b, "wv": wv_b,
            "bkq": bkq_f, "bv": bv_f,
        })
    return in_maps


_NC_CACHE = {}


def kernel(input, Wq, bq, Wk, bk, Wv, bv):
    in_maps = shard_inputs(input, Wq, bq, Wk, bk, Wv, bv)
    if "nc" not in _NC_CACHE:
        _NC_CACHE["nc"] = build()
    nc = _NC_CACHE["nc"]
    res = run_bass_kernel_spmd(nc, in_maps, core_ids=list(range(N_CORES)))
    half = S // 2
    result = np.empty((B, S, D), dtype=np.float32)
    for b in range(B):
        # core 2b: keys half0, rows in natural order
        # core 2b+1: keys half1, rows permuted [half1; half0]
        n0 = np.asarray(res.results[2 * b]["out"]).astype(np.float32).T
        d0 = np.asarray(res.results[2 * b]["den"]).astype(np.float32).ravel()
        n1p = np.asarray(res.results[2 * b + 1]["out"]).astype(np.float32).T
        d1p = np.asarray(res.results[2 * b + 1]["den"]).astype(np.float32).ravel()
        n1 = np.concatenate([n1p[half:], n1p[:half]], axis=0)
        d1 = np.concatenate([d1p[half:], d1p[:half]])
        result[b] = (n0 + n1) / (d0 + d1)[:, None]
    return result


if __name__ == "__main__":
    rng = np.random.default_rng(0)
    inputs = {
        "input": rng.standard_normal((B, S, M), dtype=np.float32),
        "Wq": (rng.standard_normal((M, D), dtype=np.float32) / np.sqrt(M)).astype(np.float32),
        "bq": (rng.standard_normal(D, dtype=np.float32) * 0.02),
        "Wk": (rng.standard_normal((M, D), dtype=np.float32) / np.sqrt(M)).astype(np.float32),
        "bk": (rng.standard_normal(D, dtype=np.float32) * 0.02),
        "Wv": (rng.standard_normal((M, D), dtype=np.float32) / np.sqrt(M)).astype(np.float32),
        "bv": (rng.standard_normal(D, dtype=np.float32) * 0.02),
    }
    out = kernel(**inputs)
    print("kernel output:", out.shape, out.dtype)


# revision 55
# speedup vs baseline: 1.4849x; 1.0126x over previous
"""Self-contained Trainium2 Bass kernel for a single attention head.

Reference computation (per batch b):
    Q = x @ Wq + bq ; K = x @ Wk + bk ; V = x @ Wv + bv      (x: [S, M])
    out = softmax(Q K^T / sqrt(D)) @ V                        ([S, D])

Shapes: B=4, S=4096, M=1024, D=128, f32.

Sharding (key-split + host merge): 8 cores; core c handles batch b=c//2 and
KEY-half h=c%2. Each core projects K/V for its own 2048 key rows only, Q for
all 4096 queries, and computes the UNNORMALIZED partial attention
  N_h^T[dv, q] = sum_{s in half h} exp(q.k_s/sqrt(D)) v_s,   d_h[q] = sum_s exp(.)
over its key half. The host merges: O = (N_0 + N_1) / (d_0 + d_1) — softmax
over the key axis is an exact sum-decomposition, so the merge is exact. This
halves the redundant K/V projection work and removes every on-device
normalization/transpose step (the host divides and transposes). The host
permutes each core's rows so its key half comes first (key order inside a
softmax is irrelevant; the query order is un-permuted on the host).

Device pipeline (fully fused stream over 8 x-chunks of 512 rows):
  - chunks 0-3 project K^T/V (own keys) + Q^T; chunks 4-7 project Q^T only.
    x is bf16 (host-converted; halves DMA). V is built in natural [s, dv]
    layout directly via x-tile-stationary matmuls (no transposes).
  - 8 q-chunks chase the stream: scores S^T[s,q] (bf16 matmuls into PSUM),
    one wide exp per s-tile pair [128,1024] -> A^T bf16, attn@V accumulates
    N^T in PSUM. Two q-chunks hold the two O-psum banks at a time; the
    others defer attn@V until a slot frees (their A^T stays in SBUF).
  - denominator: per q-chunk DVE bf16 tree -> f32 -> GPSIMD
    partition_all_reduce -> DMA; the last q-chunk accumulates its den with
    PE ones-matmuls instead so the post-last-exp tail is tiny.
  - N^T q-slabs DMA out straight from the PSUM drain; no finalize pass.
  - PSUM (8 banks): scores 2x2 + N^T accumulators 2 + projections 2.
"""

from contextlib import ExitStack

import numpy as np

import concourse.bass as bass
import concourse.tile as tile
from concourse import bacc, mybir
from concourse.bass_utils import run_bass_kernel_spmd
from concourse.masks import make_identity

F32 = mybir.dt.float32
BF16 = mybir.dt.bfloat16

B, S, M, D = 4, 4096, 1024, 128
N_CORES = 8
P = 128
SCALE = 1.0 / np.sqrt(np.float32(D))


def build_attention(nc, S_all=S, M_dim=M, SC=512, QC=512):
    KH = S_all // 2               # keys per core (2048)
    MT = M_dim // P               # m-tiles (8)
    ST = KH // P                  # key s-tiles (16)
    NSC = S_all // SC             # x-chunks (8)
    NKC = KH // SC                # key chunks (4)
    SCT = SC // P                 # s-tiles per chunk (4)
    NQC = S_all // QC             # q-chunks (8)
    NPAIR = ST // 2               # score pairs per q-chunk (8)
    LQ = NQC - 1                  # last q-chunk (PE-ones denominator)

    xT = nc.dram_tensor("xT", [M_dim, S_all], BF16, kind="ExternalInput").ap()
    wq = nc.dram_tensor("wq", [P, MT * D], BF16, kind="ExternalInput").ap()
    wk = nc.dram_tensor("wk", [P, MT * D], BF16, kind="ExternalInput").ap()
    wv = nc.dram_tensor("wv", [P, MT * D], BF16, kind="ExternalInput").ap()
    bkq = nc.dram_tensor("bkq", [P, 2], F32, kind="ExternalInput").ap()
    bv = nc.dram_tensor("bv", [1, D], F32, kind="ExternalInput").ap()
    out = nc.dram_tensor("out", [D, S_all], BF16, kind="ExternalOutput").ap()
    den = nc.dram_tensor("den", [1, S_all], F32, kind="ExternalOutput").ap()

    xT_r = xT.rearrange("(t p) s -> p t s", p=P)

    Ident = mybir.ActivationFunctionType.Identity
    Exp = mybir.ActivationFunctionType.Exp

    with tile.TileContext(nc) as tc:
        ctx = ExitStack()
        persist = ctx.enter_context(tc.tile_pool(name="persist", bufs=1))

        ident = persist.tile([P, P], F32)
        make_identity(nc, ident[:])
        identb = persist.tile([P, P], BF16)
        nc.vector.tensor_copy(identb[:], ident[:])
        ones_f = persist.tile([P, 1], F32)
        nc.vector.memset(ones_f[:], 1.0)
        onesb = persist.tile([P, 1], BF16)
        nc.vector.tensor_copy(onesb[:], ones_f[:])

        # startup DMA order: wk (gpsimd queue, instant issue), then biases +
        # x chunk 0 halves + wq/wv on the SP queue in priority order
        xstage = ctx.enter_context(tc.tile_pool(name="xstage", bufs=2))
        wk_sb = persist.tile([P, MT, D], BF16)
        nc.gpsimd.dma_start(wk_sb[:], wk.rearrange("p (t d) -> p t d", d=D))
        bkq_sb = persist.tile([P, 2], F32)
        nc.sync.dma_start(bkq_sb[:], bkq)
        bk_sb = bkq_sb[:, 0:1]
        bq_sb = bkq_sb[:, 1:2]
        bv_row = persist.tile([1, D], F32)
        nc.sync.dma_start(bv_row[:], bv)
        bv_bcast = persist.tile([P, D], F32)
        nc.gpsimd.partition_broadcast(bv_bcast[:], bv_row[:])
        x_r0 = xstage.tile([P, MT, SC], BF16, name="x_r")
        nc.sync.dma_start(x_r0[:, 0:MT // 2, :],
                          xT_r[:, 0:MT // 2, bass.ds(0, SC)])
        nc.sync.dma_start(x_r0[:, MT // 2:, :],
                          xT_r[:, MT // 2:, bass.ds(0, SC)])
        wq_sb = persist.tile([P, MT, D], BF16)
        nc.sync.dma_start(wq_sb[:], wq.rearrange("p (t d) -> p t d", d=D))
        wv_sb = persist.tile([P, MT, D], BF16)
        nc.sync.dma_start(wv_sb[:], wv.rearrange("p (t d) -> p t d", d=D))

        kT_sb = persist.tile([P, KH], BF16)        # K^T  [dk, s]
        qT_sb = persist.tile([P, S_all], BF16)     # Q^T  [dk, q]
        v_sb = persist.tile([P, ST, D], BF16)      # V    [s%128, s-tile, dv]

        apool = ctx.enter_context(tc.tile_pool(name="apool", bufs=5))
        dpool = ctx.enter_context(tc.tile_pool(name="dpool", bufs=2))
        otpool = ctx.enter_context(tc.tile_pool(name="otpool", bufs=2))
        spsum = ctx.enter_context(tc.tile_pool(name="spsum", bufs=2, space="PSUM"))
        opsum = ctx.enter_context(tc.tile_pool(name="opsum", bufs=2, space="PSUM"))

        a_t = {}      # qc -> A^T tile [P, ST, QC] bf16
        o_ps = {}     # qc -> N^T psum [P, QC]
        t1 = {}       # qc -> den partial [P, 4, QC] bf16
        dall = {}     # qc -> all-reduced partial denominator [P, QC] f32
        tail = {}     # LQ's PE-ones den psum [1, QC]

        def qsl(qc):
            return bass.ds(qc * QC, QC)

        def emit_scores(qc, pr, split_exp=False):
            """Scores for s-tiles (2pr, 2pr+1) x q-chunk qc + one wide exp."""
            if pr == 0:
                a_t[qc] = apool.tile([P, ST, QC], BF16, name="a_sb")
            ps_s = spsum.tile([P, 2, QC], F32, name="ps_s")
            for j in range(2):
                st = 2 * pr + j
                nc.tensor.matmul(ps_s[:, j, :], kT_sb[:, bass.ts(st, P)],
                                 qT_sb[:, qsl(qc)], start=True, stop=True)
            if split_exp:
                for j in range(2):
                    st = 2 * pr + j
                    nc.scalar.activation(a_t[qc][:, st:st + 1, :],
                                         ps_s[:, j:j + 1, :], Exp,
                                         scale=float(SCALE))
            else:
                nc.scalar.activation(a_t[qc][:, 2 * pr:2 * pr + 2, :], ps_s[:],
                                     Exp, scale=float(SCALE))
            # denominator tree triggers. LQ folds tiles 0..11 early (its last
            # 4 tiles ride the PE ones-matmuls so the end tail stays short).
            a = a_t[qc]
            if pr == 3:
                t1[qc] = dpool.tile([P, 4, QC], BF16, name="t1")
                nc.vector.tensor_add(t1[qc][:], a[:, 0:4, :], a[:, 4:8, :])
            elif qc != LQ and pr == 7:
                tb = dpool.tile([P, 4, QC], BF16, name="tb")
                nc.vector.tensor_add(tb[:], a[:, 8:12, :], a[:, 12:16, :])
                nc.vector.tensor_add(t1[qc][:], t1[qc][:], tb[:])
                _den_fold(qc)
                nc.sync.dma_start(den[:, qsl(qc)], dall[qc][:1, :])
            elif qc == LQ and pr == 5:
                nc.vector.tensor_add(t1[qc][:], t1[qc][:], a[:, 8:12, :])
                _den_fold(qc)

        def _den_fold(qc):
            nc.vector.tensor_add(t1[qc][:, 0:2, :], t1[qc][:, 0:2, :],
                                 t1[qc][:, 2:4, :])
            den128 = dpool.tile([P, QC], F32, name="den128")
            nc.vector.tensor_add(den128[:], t1[qc][:, 0, :], t1[qc][:, 1, :])
            dl = dpool.tile([P, QC], F32, name="dall")
            nc.gpsimd.partition_all_reduce(dl[:], den128[:], P,
                                           bass.bass_isa.ReduceOp.add)
            dall[qc] = dl

        def emit_av(qc, pr):
            """attn@V accumulation for pair pr; the last q-chunk's final four
            s-tiles also feed the PE ones-matmul denominator tail."""
            if qc not in o_ps:
                o_ps[qc] = opsum.tile([P, QC], F32, name="o_ps")
            for j in range(2):
                st = 2 * pr + j
                nc.tensor.matmul(o_ps[qc][:], v_sb[:, st, :],
                                 a_t[qc][:, st, :],
                                 start=(st == 0), stop=(st == ST - 1))
            if qc == LQ and pr >= NPAIR - 2:
                if pr == NPAIR - 2:
                    tail[qc] = opsum.tile([1, QC], F32, name="o_ps")
                for j in range(2):
                    st = 2 * pr + j
                    nc.tensor.matmul(tail[qc][:], onesb[:], a_t[qc][:, st, :],
                                     start=(st == ST - 4), stop=(st == ST - 1))

        def finish_qc(qc):
            """Drain N^T to SBUF and DMA it out; LQ also drains its PE-ones
            denominator (on ACT, parallel to the DVE drain). The last drain
            and output DMA are split in halves so the first transfer's launch
            latency overlaps the second half's drain."""
            oT = otpool.tile([P, QC], BF16, name="oT")
            if qc == LQ:
                tail_sb = dpool.tile([1, QC], F32, name="tail_sb", bufs=1)
                nc.scalar.copy(tail_sb[:], tail[qc][:])
                dcomb = dpool.tile([1, QC], F32, name="dcomb", bufs=1)
                nc.vector.tensor_add(dcomb[:], dall[qc][:1, :], tail_sb[:])
                nc.gpsimd.dma_start(den[:, qsl(qc)], dcomb[:])
                h = QC // 2
                nc.vector.tensor_copy(oT[:, 0:h], o_ps[qc][:, 0:h])
                nc.sync.dma_start(out[:, bass.ds(qc * QC, h)], oT[:, 0:h])
                nc.vector.tensor_copy(oT[:, h:], o_ps[qc][:, h:])
                # second half on the ACT queue so its launch overlaps the first
                nc.scalar.dma_start(out[:, bass.ds(qc * QC + h, h)], oT[:, h:])
            else:
                nc.vector.tensor_copy(oT[:], o_ps[qc][:])
                nc.sync.dma_start(out[:, qsl(qc)], oT[:])

        # scheduler state: two q-chunks own the O-psum banks at a time; the
        # rest chase scores only and catch up attn@V when promoted.
        sc_done = [0] * NQC
        av_done = [0] * NQC
        full = [0, 1]
        next_full = [2]

        def pump_avs(qc):
            if qc not in full:
                return
            while av_done[qc] < sc_done[qc] - (1 if sc_done[qc] < NPAIR else 0):
                emit_av(qc, av_done[qc])
                av_done[qc] += 1
            if av_done[qc] == NPAIR:
                finish_qc(qc)
                full.remove(qc)
                if next_full[0] < NQC:
                    nq = next_full[0]
                    next_full[0] += 1
                    full.append(nq)
                    pump_avs(nq)

        # PE warm-up: back-to-back transposes ramp the PE p-state while the
        # first x chunk lands.
        with tc.tile_pool(name="warm", bufs=1, space="PSUM") as wp:
            warm_ps = wp.tile([P, P], BF16, name="warm_ps")
            for _ in range(34):
                nc.tensor.transpose(warm_ps[:], identb[:], identb[:])

        with tc.tile_pool(name="pp", bufs=2, space="PSUM") as pp:
            for sc in range(NSC):
                ssl = bass.ds(sc * SC, SC)
                if sc == 0:
                    x_r = x_r0
                else:
                    x_r = xstage.tile([P, MT, SC], BF16, name="x_r")
                    nc.sync.dma_start(x_r[:], xT_r[:, :, ssl])

                if sc < NKC:
                    # K^T chunk (own key half = permuted row prefix)
                    ps = pp.tile([P, SC], F32, name="pp")
                    for mt in range(MT):
                        nc.tensor.matmul(ps[:], wk_sb[:, mt, :], x_r[:, mt, :],
                                         start=(mt == 0), stop=(mt == MT - 1))
                    nc.vector.tensor_scalar_add(kT_sb[:, ssl], ps[:], bk_sb)

                # Q^T chunk (all 8 chunks); late-chunk drains go on the DVE so
                # they don't stretch the exp cadence in the ACT-bound phase
                ps2 = pp.tile([P, SC], F32, name="pp")
                for mt in range(MT):
                    nc.tensor.matmul(ps2[:], wq_sb[:, mt, :], x_r[:, mt, :],
                                     start=(mt == 0), stop=(mt == MT - 1))
                nc.vector.tensor_scalar_add(qT_sb[:, ssl], ps2[:], bq_sb)

                # first chaser's new scores go ahead of the V projection so
                # the ACT exp queue is fed across the chunk boundary
                avail = min(NKC, sc + 1) * SCT // 2
                first = next((q for q in range(NQC)
                              if q <= sc and sc_done[q] < avail), None)
                if first is not None:
                    while sc_done[first] < avail:
                        pr = sc_done[first]
                        emit_scores(first, pr,
                                    split_exp=(first == LQ and
                                               pr == NPAIR - 1))
                        sc_done[first] += 1

                if sc < NKC:
                    # V chunk in natural [s, dv] layout: x-tile stationary
                    for t in range(SCT):
                        st = sc * SCT + t
                        psv = pp.tile([P, D], F32, name="pp")
                        for mt in range(MT):
                            nc.tensor.matmul(psv[:], x_r[:, mt, bass.ts(t, P)],
                                             wv_sb[:, mt, :],
                                             start=(mt == 0),
                                             stop=(mt == MT - 1))
                        nc.vector.tensor_add(v_sb[:, st, :], psv[:],
                                             bv_bcast[:])

                # chase: remaining q-chunks catch up; slot owners run attn@V
                for qc in range(NQC):
                    if qc <= sc:
                        while sc_done[qc] < avail:
                            pr = sc_done[qc]
                            emit_scores(qc, pr,
                                        split_exp=(qc == LQ and
                                                   pr == NPAIR - 1))
                            sc_done[qc] += 1
                            if qc in full:
                                pump_avs(qc)
                        if qc in full:
                            pump_avs(qc)

        # post-stream: everything has its scores emitted; drain the rest
        for qc in list(full):
            pump_avs(qc)
        while full:
            qc = full[0]
            pump_avs(qc)
        ctx.close()

    return nc


def build(n_cores=N_CORES, **kw):
    nc = bacc.Bacc("TRN2", target_bir_lowering=False, debug=False,
                   num_devices=n_cores)
    build_attention(nc, **kw)
    nc.compile()
    return nc


def shard_inputs(input, Wq, bq, Wk, bk, Wv, bv):
    """Per-core in_maps. Core c: batch c//2, key-half c%2; the host permutes
    the batch so the core's key rows come first, transposes to xT [M, S],
    and converts to bf16. Weights are packed [P, MT*D]; biases packed small."""
    import ml_dtypes
    half = S // 2
    MT = M // 128

    def pack_w(W):
        return np.ascontiguousarray(
            np.asarray(W, dtype=np.float32).reshape(MT, 128, D)
            .transpose(1, 0, 2).reshape(128, MT * D)).astype(ml_dtypes.bfloat16)

    wq_b, wk_b, wv_b = pack_w(Wq), pack_w(Wk), pack_w(Wv)
    bkq_f = np.ascontiguousarray(np.stack(
        [np.asarray(bk, dtype=np.float32).ravel(),
         np.asarray(bq, dtype=np.float32).ravel()], axis=1))
    bv_f = np.asarray(bv, dtype=np.float32).reshape(1, D)
    in_maps = []
    for c in range(N_CORES):
        b, h = divmod(c, 2)
        xb = np.asarray(input[b])
        x_perm = np.concatenate(
            [xb[h * half:(h + 1) * half], xb[(1 - h) * half:(2 - h) * half]],
            axis=0)
        xT = np.ascontiguousarray(x_perm.T).astype(ml_dtypes.bfloat16)
        in_maps.append({
            "xT": xT,
            "wq": wq_b, "wk": wk_b, "wv": wv_b,
            "bkq": bkq_f, "bv": bv_f,
        })
    return in_maps


_NC_CACHE = {}


def kernel(input, Wq, bq, Wk, bk, Wv, bv):
    in_maps = shard_inputs(input, Wq, bq, Wk, bk, Wv, bv)
    if "nc" not in _NC_CACHE:
        _NC_CACHE["nc"] = build()
    nc = _NC_CACHE["nc"]
    res = run_bass_kernel_spmd(nc, in_maps, core_ids=list(range(N_CORES)))
    half = S // 2
    result = np.empty((B, S, D), dtype=np.float32)
    for b in range(B):
        # core 2b: keys half0, rows in natural order
        # core 2b+1: keys half1, rows permuted [half1; half0]
        n0 = np.asarray(res.results[2 * b]["out"]).astype(np.float32).T
        d0 = np.asarray(res.results[2 * b]["den"]).astype(np.float32).ravel()
        n1p = np.asarray(res.results[2 * b + 1]["out"]).astype(np.float32).T
        d1p = np.asarray(res.results[2 * b + 1]["den"]).astype(np.float32).ravel()
        n1 = np.concatenate([n1p[half:], n1p[:half]], axis=0)
        d1 = np.concatenate([d1p[half:], d1p[:half]])
        result[b] = (n0 + n1) / (d0 + d1)[:, None]
    return result


if __name__ == "__main__":
    rng = np.random.default_rng(0)
    inputs = {
        "input": rng.standard_normal((B, S, M), dtype=np.float32),
        "Wq": (rng.standard_normal((M, D), dtype=np.float32) / np.sqrt(M)).astype(np.float32),
        "bq": (rng.standard_normal(D, dtype=np.float32) * 0.02),
        "Wk": (rng.standard_normal((M, D), dtype=np.float32) / np.sqrt(M)).astype(np.float32),
        "bk": (rng.standard_normal(D, dtype=np.float32) * 0.02),
        "Wv": (rng.standard_normal((M, D), dtype=np.float32) / np.sqrt(M)).astype(np.float32),
        "bv": (rng.standard_normal(D, dtype=np.float32) * 0.02),
    }
    out = kernel(**inputs)
    print("kernel output:", out.shape, out.dtype)


# revision 58
# speedup vs baseline: 1.4934x; 1.0058x over previous
"""Self-contained Trainium2 Bass kernel for a single attention head.

Reference computation (per batch b):
    Q = x @ Wq + bq ; K = x @ Wk + bk ; V = x @ Wv + bv      (x: [S, M])
    out = softmax(Q K^T / sqrt(D)) @ V                        ([S, D])

Shapes: B=4, S=4096, M=1024, D=128, f32.

Sharding (key-split + host merge): 8 cores; core c handles batch b=c//2 and
KEY-half h=c%2. Each core projects K/V for its own 2048 key rows only, Q for
all 4096 queries, and computes the UNNORMALIZED partial attention
  N_h^T[dv, q] = sum_{s in half h} exp(q.k_s/sqrt(D)) v_s,   d_h[q] = sum_s exp(.)
over its key half. The host merges: O = (N_0 + N_1) / (d_0 + d_1) — softmax
over the key axis is an exact sum-decomposition, so the merge is exact. This
halves the redundant K/V projection work and removes every on-device
normalization/transpose step (the host divides and transposes). The host
permutes each core's rows so its key half comes first (key order inside a
softmax is irrelevant; the query order is un-permuted on the host).

Device pipeline (fully fused stream over 8 x-chunks of 512 rows):
  - chunks 0-3 project K^T/V (own keys) + Q^T; chunks 4-7 project Q^T only.
    x is bf16 (host-converted; halves DMA). V is built in natural [s, dv]
    layout directly via x-tile-stationary matmuls (no transposes).
  - 8 q-chunks chase the stream: scores S^T[s,q] (bf16 matmuls into PSUM),
    one wide exp per s-tile pair [128,1024] -> A^T bf16, attn@V accumulates
    N^T in PSUM. Two q-chunks hold the two O-psum banks at a time; the
    others defer attn@V until a slot frees (their A^T stays in SBUF).
  - denominator: per q-chunk DVE bf16 tree -> f32 -> GPSIMD
    partition_all_reduce -> DMA; the last q-chunk accumulates its den with
    PE ones-matmuls instead so the post-last-exp tail is tiny.
  - N^T q-slabs DMA out straight from the PSUM drain; no finalize pass.
  - PSUM (8 banks): scores 2x2 + N^T accumulators 2 + projections 2.
"""

from contextlib import ExitStack

import numpy as np

import concourse.bass as bass
import concourse.tile as tile
from concourse import bacc, mybir
from concourse.bass_utils import run_bass_kernel_spmd
from concourse.masks import make_identity

F32 = mybir.dt.float32
BF16 = mybir.dt.bfloat16

B, S, M, D = 4, 4096, 1024, 128
N_CORES = 8
P = 128
SCALE = 1.0 / np.sqrt(np.float32(D))


def build_attention(nc, S_all=S, M_dim=M, SC=512, QC=512):
    KH = S_all // 2               # keys per core (2048)
    MT = M_dim // P               # m-tiles (8)
    ST = KH // P                  # key s-tiles (16)
    NSC = S_all // SC             # x-chunks (8)
    NKC = KH // SC                # key chunks (4)
    SCT = SC // P                 # s-tiles per chunk (4)
    NQC = S_all // QC             # q-chunks (8)
    NPAIR = ST // 2               # score pairs per q-chunk (8)
    LQ = NQC - 1                  # last q-chunk (PE-ones denominator)

    xT = nc.dram_tensor("xT", [M_dim, S_all], BF16, kind="ExternalInput").ap()
    wq = nc.dram_tensor("wq", [P, MT * D], BF16, kind="ExternalInput").ap()
    wk = nc.dram_tensor("wk", [P, MT * D], BF16, kind="ExternalInput").ap()
    wv = nc.dram_tensor("wv", [P, MT * D], BF16, kind="ExternalInput").ap()
    bkq = nc.dram_tensor("bkq", [P, 2], F32, kind="ExternalInput").ap()
    bv = nc.dram_tensor("bv", [1, D], F32, kind="ExternalInput").ap()
    out = nc.dram_tensor("out", [D, S_all], BF16, kind="ExternalOutput").ap()
    den = nc.dram_tensor("den", [1, S_all], F32, kind="ExternalOutput").ap()

    xT_r = xT.rearrange("(t p) s -> p t s", p=P)

    Ident = mybir.ActivationFunctionType.Identity
    Exp = mybir.ActivationFunctionType.Exp

    with tile.TileContext(nc) as tc:
        ctx = ExitStack()
        persist = ctx.enter_context(tc.tile_pool(name="persist", bufs=1))

        ident = persist.tile([P, P], F32)
        make_identity(nc, ident[:])
        identb = persist.tile([P, P], BF16)
        nc.vector.tensor_copy(identb[:], ident[:])
        ones_f = persist.tile([P, 1], F32)
        nc.vector.memset(ones_f[:], 1.0)
        onesb = persist.tile([P, 1], BF16)
        nc.vector.tensor_copy(onesb[:], ones_f[:])

        # startup DMA order: wk (gpsimd queue, instant issue), then biases +
        # x chunk 0 halves + wq/wv on the SP queue in priority order
        xstage = ctx.enter_context(tc.tile_pool(name="xstage", bufs=2))
        wk_sb = persist.tile([P, MT, D], BF16)
        nc.gpsimd.dma_start(wk_sb[:], wk.rearrange("p (t d) -> p t d", d=D))
        bkq_sb = persist.tile([P, 2], F32)
        nc.scalar.dma_start(bkq_sb[:], bkq)
        bk_sb = bkq_sb[:, 0:1]
        bq_sb = bkq_sb[:, 1:2]
        bv_row = persist.tile([1, D], F32)
        nc.scalar.dma_start(bv_row[:], bv)
        bv_bcast = persist.tile([P, D], F32)
        nc.gpsimd.partition_broadcast(bv_bcast[:], bv_row[:])
        x_r0 = xstage.tile([P, MT, SC], BF16, name="x_r")
        nc.sync.dma_start(x_r0[:, 0:MT // 2, :],
                          xT_r[:, 0:MT // 2, bass.ds(0, SC)])
        nc.sync.dma_start(x_r0[:, MT // 2:, :],
                          xT_r[:, MT // 2:, bass.ds(0, SC)])
        wq_sb = persist.tile([P, MT, D], BF16)
        nc.sync.dma_start(wq_sb[:], wq.rearrange("p (t d) -> p t d", d=D))
        wv_sb = persist.tile([P, MT, D], BF16)
        nc.sync.dma_start(wv_sb[:], wv.rearrange("p (t d) -> p t d", d=D))

        kT_sb = persist.tile([P, KH], BF16)        # K^T  [dk, s]
        qT_sb = persist.tile([P, S_all], BF16)     # Q^T  [dk, q]
        v_sb = persist.tile([P, ST, D], BF16)      # V    [s%128, s-tile, dv]

        apool = ctx.enter_context(tc.tile_pool(name="apool", bufs=5))
        dpool = ctx.enter_context(tc.tile_pool(name="dpool", bufs=2))
        otpool = ctx.enter_context(tc.tile_pool(name="otpool", bufs=2))
        spsum = ctx.enter_context(tc.tile_pool(name="spsum", bufs=2, space="PSUM"))
        opsum = ctx.enter_context(tc.tile_pool(name="opsum", bufs=2, space="PSUM"))

        a_t = {}      # qc -> A^T tile [P, ST, QC] bf16
        o_ps = {}     # qc -> N^T psum [P, QC]
        t1 = {}       # qc -> den partial [P, 4, QC] bf16
        dall = {}     # qc -> all-reduced partial denominator [P, QC] f32
        tail = {}     # LQ's PE-ones den psum [1, QC]

        def qsl(qc):
            return bass.ds(qc * QC, QC)

        def emit_scores(qc, pr, split_exp=False):
            """Scores for s-tiles (2pr, 2pr+1) x q-chunk qc + one wide exp."""
            if pr == 0:
                a_t[qc] = apool.tile([P, ST, QC], BF16, name="a_sb")
            ps_s = spsum.tile([P, 2, QC], F32, name="ps_s")
            for j in range(2):
                st = 2 * pr + j
                nc.tensor.matmul(ps_s[:, j, :], kT_sb[:, bass.ts(st, P)],
                                 qT_sb[:, qsl(qc)], start=True, stop=True)
            if split_exp:
                for j in range(2):
                    st = 2 * pr + j
                    nc.scalar.activation(a_t[qc][:, st:st + 1, :],
                                         ps_s[:, j:j + 1, :], Exp,
                                         scale=float(SCALE))
            else:
                nc.scalar.activation(a_t[qc][:, 2 * pr:2 * pr + 2, :], ps_s[:],
                                     Exp, scale=float(SCALE))
            # denominator tree triggers. LQ folds tiles 0..11 early (its last
            # 4 tiles ride the PE ones-matmuls so the end tail stays short).
            a = a_t[qc]
            if pr == 3:
                t1[qc] = dpool.tile([P, 4, QC], BF16, name="t1")
                nc.vector.tensor_add(t1[qc][:], a[:, 0:4, :], a[:, 4:8, :])
            elif qc != LQ and pr == 7:
                tb = dpool.tile([P, 4, QC], BF16, name="tb")
                nc.vector.tensor_add(tb[:], a[:, 8:12, :], a[:, 12:16, :])
                nc.vector.tensor_add(t1[qc][:], t1[qc][:], tb[:])
                _den_fold(qc)
                nc.sync.dma_start(den[:, qsl(qc)], dall[qc][:1, :])
            elif qc == LQ and pr == 5:
                nc.vector.tensor_add(t1[qc][:], t1[qc][:], a[:, 8:12, :])
                _den_fold(qc)

        def _den_fold(qc):
            nc.vector.tensor_add(t1[qc][:, 0:2, :], t1[qc][:, 0:2, :],
                                 t1[qc][:, 2:4, :])
            den128 = dpool.tile([P, QC], F32, name="den128")
            nc.vector.tensor_add(den128[:], t1[qc][:, 0, :], t1[qc][:, 1, :])
            dl = dpool.tile([P, QC], F32, name="dall")
            nc.gpsimd.partition_all_reduce(dl[:], den128[:], P,
                                           bass.bass_isa.ReduceOp.add)
            dall[qc] = dl

        def emit_av(qc, pr):
            """attn@V accumulation for pair pr; the last q-chunk's final four
            s-tiles also feed the PE ones-matmul denominator tail."""
            if qc not in o_ps:
                o_ps[qc] = opsum.tile([P, QC], F32, name="o_ps")
            for j in range(2):
                st = 2 * pr + j
                nc.tensor.matmul(o_ps[qc][:], v_sb[:, st, :],
                                 a_t[qc][:, st, :],
                                 start=(st == 0), stop=(st == ST - 1))
            if qc == LQ and pr >= NPAIR - 2:
                if pr == NPAIR - 2:
                    tail[qc] = opsum.tile([1, QC], F32, name="o_ps")
                for j in range(2):
                    st = 2 * pr + j
                    nc.tensor.matmul(tail[qc][:], onesb[:], a_t[qc][:, st, :],
                                     start=(st == ST - 4), stop=(st == ST - 1))

        def finish_qc(qc):
            """Drain N^T to SBUF and DMA it out; LQ also drains its PE-ones
            denominator (on ACT, parallel to the DVE drain). The last drain
            and output DMA are split in halves so the first transfer's launch
            latency overlaps the second half's drain."""
            oT = otpool.tile([P, QC], BF16, name="oT")
            if qc == LQ:
                tail_sb = dpool.tile([1, QC], F32, name="tail_sb", bufs=1)
                nc.scalar.copy(tail_sb[:], tail[qc][:])
                dcomb = dpool.tile([1, QC], F32, name="dcomb", bufs=1)
                nc.vector.tensor_add(dcomb[:], dall[qc][:1, :], tail_sb[:])
                nc.gpsimd.dma_start(den[:, qsl(qc)], dcomb[:])
                h = QC // 2
                nc.vector.tensor_copy(oT[:, 0:h], o_ps[qc][:, 0:h])
                nc.sync.dma_start(out[:, bass.ds(qc * QC, h)], oT[:, 0:h])
                nc.vector.tensor_copy(oT[:, h:], o_ps[qc][:, h:])
                # second half on the ACT queue so its launch overlaps the first
                nc.scalar.dma_start(out[:, bass.ds(qc * QC + h, h)], oT[:, h:])
            else:
                nc.vector.tensor_copy(oT[:], o_ps[qc][:])
                nc.sync.dma_start(out[:, qsl(qc)], oT[:])

        # scheduler state: two q-chunks own the O-psum banks at a time; the
        # rest chase scores only and catch up attn@V when promoted.
        sc_done = [0] * NQC
        av_done = [0] * NQC
        full = [0, 1]
        next_full = [2]

        def pump_avs(qc):
            if qc not in full:
                return
            while av_done[qc] < sc_done[qc] - (1 if sc_done[qc] < NPAIR else 0):
                emit_av(qc, av_done[qc])
                av_done[qc] += 1
            if av_done[qc] == NPAIR:
                finish_qc(qc)
                full.remove(qc)
                if next_full[0] < NQC:
                    nq = next_full[0]
                    next_full[0] += 1
                    full.append(nq)
                    pump_avs(nq)

        # PE warm-up: back-to-back transposes ramp the PE p-state while the
        # first x chunk lands.
        with tc.tile_pool(name="warm", bufs=1, space="PSUM") as wp:
            warm_ps = wp.tile([P, P], BF16, name="warm_ps")
            for _ in range(28):
                nc.tensor.transpose(warm_ps[:], identb[:], identb[:])

        with tc.tile_pool(name="pp", bufs=2, space="PSUM") as pp:
            for sc in range(NSC):
                ssl = bass.ds(sc * SC, SC)
                if sc == 0:
                    x_r = x_r0
                else:
                    x_r = xstage.tile([P, MT, SC], BF16, name="x_r")
                    nc.sync.dma_start(x_r[:], xT_r[:, :, ssl])

                if sc < NKC:
                    # K^T chunk (own key half = permuted row prefix)
                    ps = pp.tile([P, SC], F32, name="pp")
                    for mt in range(MT):
                        nc.tensor.matmul(ps[:], wk_sb[:, mt, :], x_r[:, mt, :],
                                         start=(mt == 0), stop=(mt == MT - 1))
                    nc.vector.tensor_scalar_add(kT_sb[:, ssl], ps[:], bk_sb)

                # Q^T chunk (all 8 chunks); late-chunk drains go on the DVE so
                # they don't stretch the exp cadence in the ACT-bound phase
                ps2 = pp.tile([P, SC], F32, name="pp")
                for mt in range(MT):
                    nc.tensor.matmul(ps2[:], wq_sb[:, mt, :], x_r[:, mt, :],
                                     start=(mt == 0), stop=(mt == MT - 1))
                nc.vector.tensor_scalar_add(qT_sb[:, ssl], ps2[:], bq_sb)

                # first chaser's new scores go ahead of the V projection so
                # the ACT exp queue is fed across the chunk boundary
                avail = min(NKC, sc + 1) * SCT // 2
                first = next((q for q in range(NQC)
                              if q <= sc and sc_done[q] < avail), None)
                if first is not None:
                    while sc_done[first] < avail:
                        pr = sc_done[first]
                        emit_scores(first, pr,
                                    split_exp=(first == LQ and
                                               pr == NPAIR - 1))
                        sc_done[first] += 1

                if sc < NKC:
                    # V chunk in natural [s, dv] layout: x-tile stationary
                    for t in range(SCT):
                        st = sc * SCT + t
                        psv = pp.tile([P, D], F32, name="pp")
                        for mt in range(MT):
                            nc.tensor.matmul(psv[:], x_r[:, mt, bass.ts(t, P)],
                                             wv_sb[:, mt, :],
                                             start=(mt == 0),
                                             stop=(mt == MT - 1))
                        nc.vector.tensor_add(v_sb[:, st, :], psv[:],
                                             bv_bcast[:])

                # chase: remaining q-chunks catch up; slot owners run attn@V
                for qc in range(NQC):
                    if qc <= sc:
                        while sc_done[qc] < avail:
                            pr = sc_done[qc]
                            emit_scores(qc, pr,
                                        split_exp=(qc == LQ and
                                                   pr == NPAIR - 1))
                            sc_done[qc] += 1
                            if qc in full:
                                pump_avs(qc)
                        if qc in full:
                            pump_avs(qc)

        # post-stream: everything has its scores emitted; drain the rest
        for qc in list(full):
            pump_avs(qc)
        while full:
            qc = full[0]
            pump_avs(qc)
        ctx.close()

    return nc


def build(n_cores=N_CORES, **kw):
    nc = bacc.Bacc("TRN2", target_bir_lowering=False, debug=False,
                   num_devices=n_cores)
    build_attention(nc, **kw)
    nc.compile()
    return nc


def shard_inputs(input, Wq, bq, Wk, bk, Wv, bv):
    """Per-core in_maps. Core c: batch c//2, key-half c%2; the host permutes
    the batch so the core's key rows come first, transposes to xT [M, S],
    and converts to bf16. Weights are packed [P, MT*D]; biases packed small."""
    import ml_dtypes
    half = S // 2
    MT = M // 128

    def pack_w(W):
        return np.ascontiguousarray(
            np.asarray(W, dtype=np.float32).reshape(MT, 128, D)
            .transpose(1, 0, 2).reshape(128, MT * D)).astype(ml_dtypes.bfloat16)

    wq_b, wk_b, wv_b = pack_w(Wq), pack_w(Wk), pack_w(Wv)
    bkq_f = np.ascontiguousarray(np.stack(
        [np.asarray(bk, dtype=np.float32).ravel(),
         np.asarray(bq, dtype=np.float32).ravel()], axis=1))
    bv_f = np.asarray(bv, dtype=np.float32).reshape(1, D)
    in_maps = []
    for c in range(N_CORES):
        b, h = divmod(c, 2)
        xb = np.asarray(input[b])
        x_perm = np.concatenate(
            [xb[h * half:(h + 1) * half], xb[(1 - h) * half:(2 - h) * half]],
            axis=0)
        xT = np.ascontiguousarray(x_perm.T).astype(ml_dtypes.bfloat16)
        in_maps.append({
            "xT": xT,
            "wq": wq_b, "wk": wk_b, "wv": wv_b,
            "bkq": bkq_f, "bv": bv_f,
        })
    return in_maps


_NC_CACHE = {}


def kernel(input, Wq, bq, Wk, bk, Wv, bv):
    in_maps = shard_inputs(input, Wq, bq, Wk, bk, Wv, bv)
    if "nc" not in _NC_CACHE:
        _NC_CACHE["nc"] = build()
    nc = _NC_CACHE["nc"]
    res = run_bass_kernel_spmd(nc, in_maps, core_ids=list(range(N_CORES)))
    half = S // 2
    result = np.empty((B, S, D), dtype=np.float32)
    for b in range(B):
        # core 2b: keys half0, rows in natural order
        # core 2b+1: keys half1, rows permuted [half1; half0]
        n0 = np.asarray(res.results[2 * b]["out"]).astype(np.float32).T
        d0 = np.asarray(res.results[2 * b]["den"]).astype(np.float32).ravel()
        n1p = np.asarray(res.results[2 * b + 1]["out"]).astype(np.float32).T
        d1p = np.asarray(res.results[2 * b + 1]["den"]).astype(np.float32).ravel()
        n1 = np.concatenate([n1p[half:], n1p[:half]], axis=0)
        d1 = np.concatenate([d1p[half:], d1p[:half]])
        result[b] = (n0 + n1) / (d0 + d1)[:, None]
    return result


if __name__ == "__main__":
    rng = np.random.default_rng(0)
    inputs = {
        "input": rng.standard_normal((B, S, M), dtype=np.float32),
        "Wq": (rng.standard_normal((M, D), dtype=np.float32) / np.sqrt(M)).astype(np.float32),
        "bq": (rng.standard_normal(D, dtype=np.float32) * 0.02),
        "Wk": (rng.standard_normal((M, D), dtype=np.float32) / np.sqrt(M)).astype(np.float32),
        "bk": (rng.standard_normal(D, dtype=np.float32) * 0.02),
        "Wv": (rng.standard_normal((M, D), dtype=np.float32) / np.sqrt(M)).astype(np.float32),
        "bv": (rng.standard_normal(D, dtype=np.float32) * 0.02),
    }
    out = kernel(**inputs)
    print("kernel output:", out.shape, out.dtype)
